# revision 1
# baseline (speedup 1.0000x reference)
"""Bass/Trainium2 kernel for nn_BitGatConv (GAT-style message passing).

Self-contained: takes full inputs, shards edges by destination window across
8 NeuronCores (SPMD, one program), returns the full [N, HC] output.

Algorithm (per core, rotated node ids so all cores run the same program):
  Phase A (build): h = nodes_ft @ W, att_j = nodes_ft @ (W@A2),
    att_i = nodes_ft @ (W@A1); store bf16 tables
      hj_table [N_PAD, 128]  rows = [h | att_j]
      ao_table [NSHARD+1, 128] rows = [att_i | onehot64(node mod 64)]
      (row NSHARD = sentinel: att_i = -1e4 so exp()==0 for pad edges)
  Phase B (edges): for each 128-edge bin, gather hj rows by src and ao rows
    by local tgt; s = att_i + att_j; l = max(0.2*s, s); x = exp(l);
    payload = [x*h | x]; one-hot matmul accumulates [numer | denom] into a
    per-64-node-window PSUM tile (K bins per window, K uniform).
    No segment-max subtraction: logits are bounded (~|s|<10) so exp is safe,
    and softmax is shift-free identical.
  Phase C (flush): out = numer / (denom + 1e-16) + bias.
"""

import math
import os
import sys
from contextlib import ExitStack

import numpy as np

for _p in ("/opt/trn_rl_repo",):
    if _p not in sys.path:
        sys.path.insert(0, _p)

import ml_dtypes  # noqa: E402

BF16_NP = ml_dtypes.bfloat16

# ---------------------------------------------------------------------------
# Problem constants (hardcoded per contest rules)
N_NODES = 50000
N_EDGES = 800000
IN_CH = 128
HC = 64
NEG_SLOPE = 0.2
N_CORES = 8
W_WIN = 64  # nodes per scatter window (one-hot width)
SENT_ATT = -10000.0


def _cfg(n_nodes, n_edges, n_cores=N_CORES, w=W_WIN):
    nw = math.ceil(n_nodes / w)
    npc = math.ceil(nw / n_cores)  # windows per core
    if npc % 2 == 1:
        npc += 1  # need even (flush in pairs)
    n_pad = n_cores * npc * w
    nshard = npc * w
    # group_nw: windows per gather-group (batch for gathers/DVE)
    group_nw = 1
    for cand in (7, 6, 5, 4, 8, 3, 2):
        if npc % cand == 0:
            group_nw = cand
            break
    return dict(
        N=n_nodes, E=n_edges, NC=n_cores, W=w, NPC=npc,
        N_PAD=n_pad, NSHARD=nshard, GROUP_NW=group_nw,
        T_TILES=n_pad // 128, SHARD_TILES=nshard // 128,
    )


def _prep(inputs, cfg):
    """Host-side preprocessing: shard + pad + index building (numpy only)."""
    N, E, NC, W = cfg["N"], cfg["E"], cfg["NC"], cfg["W"]
    NPC, N_PAD, NSHARD = cfg["NPC"], cfg["N_PAD"], cfg["NSHARD"]

    nodes_ft = np.asarray(inputs["nodes_ft"], dtype=np.float32)
    adj = np.asarray(inputs["adj_list"])
    weight = np.asarray(inputs["weight"], dtype=np.float32)
    a1 = np.asarray(inputs["att_layer_1"], dtype=np.float32)
    a2 = np.asarray(inputs["att_layer_2"], dtype=np.float32)
    bias = np.asarray(inputs["bias"], dtype=np.float32)

    tgt = adj[0].astype(np.int64)
    src = adj[1].astype(np.int64)

    win = tgt // W
    core = win // NPC
    wloc = win % NPC
    GW = cfg["GROUP_NW"]
    HL = N_PAD // 2  # hj table split point (int16 index reach)

    src_rot = (src - core * NSHARD) % N_PAD
    half = (src_rot >= HL).astype(np.int64)  # 0 = lo table, 1 = hi table

    grp = win * 2 + half
    cnt2 = np.bincount(grp, minlength=NC * NPC * 2)
    KL = max(1, int(math.ceil(cnt2[0::2].max() / 128.0)))
    KH = max(1, int(math.ceil(cnt2[1::2].max() / 128.0)))
    K = KL + KH
    B = NPC * K  # bins per core
    NB = GW * K  # bins per gather group
    ngroups = NPC // GW

    order = np.argsort(grp, kind="stable")
    starts = np.zeros(NC * NPC * 2 + 1, dtype=np.int64)
    starts[1:] = np.cumsum(cnt2)
    rank = np.arange(E, dtype=np.int64) - starts[grp[order]]

    eo = order
    c_e = core[eo]
    wl = wloc[eo]
    g_e = wl // GW
    wlg = wl % GW
    h_e = half[eo]
    j_e = rank // 128
    p_e = rank % 128
    # bin index within core: group-major, [GW windows' lo bins | GW hi bins]
    b_e = g_e * NB + np.where(
        h_e == 0, wlg * KL + j_e, GW * KL + wlg * KH + j_e)

    # int16 idx streams in dma_gather wrapped layout (idx i -> [i%16, i//16])
    def wrap16(stream2d):
        # stream2d: [NC, L] -> [NC, 128, L//16]
        ncc, L = stream2d.shape
        w = stream2d.reshape(ncc, L // 16, 16).transpose(0, 2, 1)
        return np.ascontiguousarray(np.tile(w, (1, 8, 1)))

    ao_s = np.full((NC, B * 128), NSHARD, dtype=np.int16)
    ao_s[c_e, b_e * 128 + p_e] = (tgt[eo] - c_e * NSHARD).astype(np.int16)

    # lo/hi bin serial numbers within core (for the per-half gather streams)
    lob_e = g_e * (GW * KL) + wlg * KL + j_e
    hib_e = g_e * (GW * KH) + wlg * KH + j_e
    lo_s = np.zeros((NC, NPC * KL * 128), dtype=np.int16)
    hi_s = np.zeros((NC, NPC * KH * 128), dtype=np.int16)
    m0 = h_e == 0
    lo_s[c_e[m0], lob_e[m0] * 128 + p_e[m0]] = src_rot[eo][m0].astype(np.int16)
    m1 = ~m0
    hi_s[c_e[m1], hib_e[m1] * 128 + p_e[m1]] = (
        src_rot[eo][m1] - HL).astype(np.int16)

    ao_idx = wrap16(ao_s)
    lo_idx = wrap16(lo_s)
    hi_idx = wrap16(hi_s)

    # rotated, transposed, padded node features (bf16)
    base = np.zeros((IN_CH, N_PAD), dtype=np.float32)
    base[:, :N] = nodes_ft.T

    wh = weight.astype(BF16_NP)
    wi = (weight @ a1).astype(BF16_NP)
    wj = (weight @ a2).astype(BF16_NP)

    oh = np.zeros((NSHARD + 1, HC), dtype=np.float32)
    oh[np.arange(NSHARD), np.arange(NSHARD) % W] = 1.0
    # wide windows (W < HC unused cols stay 0); sentinel points at slot 0
    oh[NSHARD, 0] = 1.0
    oh = oh.astype(BF16_NP)

    sent_row = np.full((1, HC), SENT_ATT, dtype=np.float32).astype(BF16_NP)

    npair = NPC // 2
    bias_full = np.tile(bias[None, :], (128, npair)).astype(np.float32)

    in_maps = []
    for c in range(NC):
        nftT = np.ascontiguousarray(np.roll(base, -c * NSHARD, axis=1))
        in_maps.append({
            "nodes_ftT": nftT.astype(BF16_NP),
            "wh": wh, "wi": wi, "wj": wj,
            "onehot_const": oh,
            "sent_row": sent_row,
            "lo_idx": lo_idx[c],
            "hi_idx": hi_idx[c],
            "ao_idx": ao_idx[c],
            "bias_bc": bias_full,
        })
    meta = dict(K=K, KL=KL, KH=KH, B=B)
    return in_maps, meta


def _build_program(cfg, K, KL, KH, debug_dump=False, phase_limit="full",
                   repeat=1):
    import concourse.bacc as bacc
    import concourse.bass as bass
    import concourse.mybir as mybir
    import concourse.tile as tile

    BF16 = mybir.dt.bfloat16
    F32 = mybir.dt.float32
    I16 = mybir.dt.int16
    ALU = mybir.AluOpType
    ACT = mybir.ActivationFunctionType

    NPC, N_PAD, NSHARD = cfg["NPC"], cfg["N_PAD"], cfg["NSHARD"]
    T_TILES, SHARD_TILES = cfg["T_TILES"], cfg["SHARD_TILES"]
    GROUP_NW = cfg["GROUP_NW"]
    assert K == KL + KH
    B = NPC * K
    NB = GROUP_NW * K          # bins per gather group
    NBL = GROUP_NW * KL        # lo bins per group
    NBH = GROUP_NW * KH
    NGROUPS = NPC // GROUP_NW
    NPAIR = NPC // 2
    HL = N_PAD // 2

    nc = bacc.Bacc("TRN2", target_bir_lowering=False, debug=False,
                   num_swdge_queues=4)

    nodes_ftT = nc.dram_tensor("nodes_ftT", [IN_CH, N_PAD], BF16, kind="ExternalInput")
    wh_d = nc.dram_tensor("wh", [IN_CH, HC], BF16, kind="ExternalInput")
    wi_d = nc.dram_tensor("wi", [IN_CH, HC], BF16, kind="ExternalInput")
    wj_d = nc.dram_tensor("wj", [IN_CH, HC], BF16, kind="ExternalInput")
    oh_d = nc.dram_tensor("onehot_const", [NSHARD + 1, HC], BF16, kind="ExternalInput")
    sent_d = nc.dram_tensor("sent_row", [1, HC], BF16, kind="ExternalInput")
    loidx_d = nc.dram_tensor("lo_idx", [128, NPC * KL * 8], I16, kind="ExternalInput")
    hiidx_d = nc.dram_tensor("hi_idx", [128, NPC * KH * 8], I16, kind="ExternalInput")
    aoidx_d = nc.dram_tensor("ao_idx", [128, NPC * K * 8], I16, kind="ExternalInput")
    bias_d = nc.dram_tensor("bias_bc", [128, NPAIR * HC], F32, kind="ExternalInput")
    out_d = nc.dram_tensor("out", [NSHARD, HC], F32, kind="ExternalOutput")

    hj_table = nc.dram_tensor("hj_table", [N_PAD, 2 * HC], BF16, kind="Internal")
    ao_table = nc.dram_tensor("ao_table", [NSHARD + 1, 2 * HC], BF16, kind="Internal")

    do_build = phase_limit != "noop"
    do_gather = phase_limit in ("gather", "nomm", "full")
    do_dve = phase_limit in ("nomm", "full")
    do_mm = phase_limit == "full"

    with tile.TileContext(nc) as tc, ExitStack() as ctx:
        const_pool = ctx.enter_context(tc.tile_pool(name="const", bufs=1))
        b_in = ctx.enter_context(tc.tile_pool(name="b_in", bufs=4))
        b_ps = ctx.enter_context(tc.tile_pool(name="b_ps", bufs=2, space="PSUM"))
        b_st = ctx.enter_context(tc.tile_pool(name="b_st", bufs=4))
        idx_pool = ctx.enter_context(tc.tile_pool(name="idx", bufs=4))
        g_pool = ctx.enter_context(tc.tile_pool(name="gp", bufs=2))
        ao_pool = ctx.enter_context(tc.tile_pool(name="aop", bufs=2))
        s_pool = ctx.enter_context(tc.tile_pool(name="sp", bufs=2))
        mm_ps = ctx.enter_context(tc.tile_pool(name="mmps", bufs=4, space="PSUM"))
        fl_pool = ctx.enter_context(tc.tile_pool(name="fl", bufs=1))

        wh_sb = const_pool.tile([IN_CH, HC], BF16)
        nc.sync.dma_start(wh_sb[:], wh_d[:])
        wi_sb = const_pool.tile([IN_CH, HC], BF16)
        nc.sync.dma_start(wi_sb[:], wi_d[:])
        wj_sb = const_pool.tile([IN_CH, HC], BF16)
        nc.sync.dma_start(wj_sb[:], wj_d[:])
        bias_sb = const_pool.tile([128, NPAIR * HC], F32)
        nc.sync.dma_start(bias_sb[:], bias_d[:])

        # constant halves of ao_table (DRAM->DRAM)
        nc.sync.dma_start(ao_table[:, HC:2 * HC], oh_d[:])
        nc.sync.dma_start(ao_table[NSHARD:NSHARD + 1, 0:HC], sent_d[:])

        def emit_once(rep):
            # ---- Phase A: build tables (replicated on every core)
            # two node-tiles per iteration: batched DMAs, alternating HWDGE
            # engines (sync / scalar are separate HW-DGE rings)
            for t2 in range(T_TILES // 2 if do_build else 0):
                t = 2 * t2
                dmae = nc.sync if t2 % 2 == 0 else nc.scalar
                nf = b_in.tile([128, 2, 128], BF16, name=f"nf")
                dmae.dma_start(
                    nf[:].rearrange("p a b -> p (a b)"),
                    nodes_ftT[:, 128 * t:128 * (t + 2)])
                ps = b_ps.tile([128, 2, 2 * HC], F32, name=f"bps")
                for u in range(2):
                    nc.tensor.matmul(ps[:, u, 0:HC], nf[:, u, :], wh_sb[:],
                                     start=(u == 0), stop=False)
                    nc.tensor.matmul(ps[:, u, HC:2 * HC], nf[:, u, :], wj_sb[:],
                                     start=False, stop=(u == 1))
                st = b_st.tile([128, 2, 2 * HC], BF16, name=f"bst")
                if t2 % 2 == 0:
                    nc.vector.tensor_copy(st[:], ps[:])
                else:
                    nc.scalar.copy(st[:], ps[:])
                dmae.dma_start(
                    hj_table[128 * t:128 * (t + 2), :].rearrange(
                        "(a p) b -> p a b", p=128),
                    st[:])
            # att_i shard tiles (first SHARD_TILES node-tiles, done separately
            # so hj batching stays uniform)
            for t in range(SHARD_TILES if do_build else 0):
                nf2 = b_in.tile([128, 128], BF16, tag="nf2", name="nf2")
                dmae = nc.scalar if t % 2 == 0 else nc.sync
                dmae.dma_start(nf2[:], nodes_ftT[:, 128 * t:128 * (t + 1)])
                ps2 = b_ps.tile([128, HC], F32, tag="bps2", name="bps2")
                nc.tensor.matmul(ps2[:], nf2[:], wi_sb[:], start=True, stop=True)
                sa = b_st.tile([128, HC], BF16, tag="sa", name="sa")
                if t % 2 == 0:
                    nc.scalar.copy(sa[:], ps2[:])
                else:
                    nc.vector.tensor_copy(sa[:], ps2[:])
                dmae.dma_start(ao_table[128 * t:128 * (t + 1), 0:HC], sa[:])

            if int(os.environ.get("GAT_BARRIER", "0")):
                tc.strict_bb_all_engine_barrier()

            # ---- Phase B: edge processing
            stage_n = fl_pool.tile([128, NPAIR * HC], F32, tag="sn", name="sn")
            stage_d = fl_pool.tile([128, NPAIR * HC], F32, tag="sd", name="sd")

            pair_tiles = {}
            last_G = last_AO = None
            for g in range(NGROUPS if do_gather else 0):
                sl = idx_pool.tile([128, NBL * 8], I16, tag="sl", name="sl")
                nc.sync.dma_start(sl[:], loidx_d[:, g * NBL * 8:(g + 1) * NBL * 8])
                sh = idx_pool.tile([128, NBH * 8], I16, tag="sh", name="sh")
                nc.sync.dma_start(sh[:], hiidx_d[:, g * NBH * 8:(g + 1) * NBH * 8])
                ai = idx_pool.tile([128, NB * 8], I16, tag="ai", name="ai")
                nc.sync.dma_start(ai[:], aoidx_d[:, g * NB * 8:(g + 1) * NB * 8])

                G = g_pool.tile([128, NB, 2 * HC], BF16, tag="G", name="G")
                AOt = ao_pool.tile([128, NB, 2 * HC], BF16, tag="AO", name="AOt")
                qn = 0

                def chunked_gather(out_tile, table_ap, idx_tile, nbins, parts):
                    nonlocal qn
                    cuts = [nbins * i // parts for i in range(parts + 1)]
                    for a, b2 in zip(cuts[:-1], cuts[1:]):
                        if a == b2:
                            continue
                        nc.gpsimd.dma_gather(
                            out_ap=out_tile[:, a:b2, :], in_ap=table_ap,
                            idxs_ap=idx_tile[:, a * 8:b2 * 8],
                            num_idxs=(b2 - a) * 128,
                            num_idxs_reg=(b2 - a) * 128,
                            elem_size=2 * HC, queue_num=qn % 4,
                            single_packet=False,
                        )
                        qn += 1

                chunked_gather(G[:, 0:NBL, :].rearrange("p a b -> p a b"),
                               hj_table[0:HL, :], sl, NBL, 2)
                chunked_gather(G[:, NBL:NB, :].rearrange("p a b -> p a b"),
                               hj_table[HL:N_PAD, :], sh, NBH, 2)
                chunked_gather(AOt[:], ao_table[:], ai, NB, 4)
                last_G, last_AO = G, AOt

                if not do_dve:
                    continue
                S = s_pool.tile([128, NB, HC], BF16, tag="S", name="S")
                # s = att_j + att_i
                nc.vector.tensor_tensor(
                    out=S[:], in0=G[:, :, HC:2 * HC], in1=AOt[:, :, 0:HC], op=ALU.add)
                # l = max(0.2*s, s)  (leaky relu)
                nc.vector.scalar_tensor_tensor(
                    out=S[:], in0=S[:], scalar=NEG_SLOPE, in1=S[:],
                    op0=ALU.mult, op1=ALU.max)
                # x = exp(l) -> overwrite att_j half of G
                nc.scalar.activation(G[:, :, HC:2 * HC], S[:], ACT.Exp)
                # y = h * x -> overwrite h half of G
                nc.vector.tensor_tensor(
                    out=G[:, :, 0:HC], in0=G[:, :, 0:HC], in1=G[:, :, HC:2 * HC],
                    op=ALU.mult)

                for bl in range(NB if do_mm else 0):
                    if bl < NBL:
                        w = g * GROUP_NW + bl // KL
                        j = bl % KL
                    else:
                        l2 = bl - NBL
                        w = g * GROUP_NW + l2 // KH
                        j = KL + l2 % KH
                    pr, half = w // 2, w % 2
                    if j == 0 and half == 0:
                        pair_tiles[pr] = mm_ps.tile(
                            [128, 2 * HC], F32, tag="pp", name=f"pp{pr}")
                    ps_t = pair_tiles[pr]
                    nc.tensor.matmul(
                        ps_t[HC * half:HC * half + HC, :],
                        AOt[:, bl, HC:2 * HC],
                        G[:, bl, :],
                        start=(j == 0), stop=(j == K - 1),
                        tile_position=(0, HC * half),
                        skip_group_check=True,
                    )
                    if j == K - 1 and half == 1:
                        nc.vector.tensor_copy(
                            stage_n[:, HC * pr:HC * (pr + 1)], ps_t[:, 0:HC])
                        nc.vector.tensor_copy(
                            stage_d[:, HC * pr:HC * (pr + 1)], ps_t[:, HC:2 * HC])
                        del pair_tiles[pr]

            # ---- Phase C: out = numer / (denom + eps) + bias
            if not do_mm:
                nc.vector.memset(stage_n[:], 0.0)
                nc.vector.memset(stage_d[:], 1.0)
            nc.vector.tensor_scalar_add(stage_d[:], stage_d[:], 1e-16)
            lnd = fl_pool.tile([128, NPAIR * HC], F32, tag="lnd", name="lnd")
            nc.scalar.activation(lnd[:], stage_d[:], ACT.Ln)
            nc.scalar.activation(lnd[:], lnd[:], ACT.Exp, scale=-1.0)
            nc.vector.tensor_tensor(out=stage_n[:], in0=stage_n[:], in1=lnd[:],
                                    op=ALU.mult)
            nc.vector.tensor_tensor(out=stage_n[:], in0=stage_n[:], in1=bias_sb[:],
                                    op=ALU.add)

            out_view = out_d[:].rearrange("(pr p) c -> p pr c", p=128)
            st_view = stage_n[:].rearrange("p (pr c) -> p pr c", c=HC)
            nc.sync.dma_start(out_view, st_view)
            return last_G, last_AO, stage_d

        for rep in range(repeat):
            last_G, last_AO, stage_d = emit_once(rep)
            if repeat > 1:
                tc.strict_bb_all_engine_barrier()

        if debug_dump:
            dump_hj = nc.dram_tensor("dump_hj", [N_PAD, 2 * HC], BF16,
                                     kind="ExternalOutput")
            dump_ao = nc.dram_tensor("dump_ao", [NSHARD + 1, 2 * HC], BF16,
                                     kind="ExternalOutput")
            dump_sd = nc.dram_tensor("dump_sd", [128, NPAIR * HC], F32,
                                     kind="ExternalOutput")
            dump_g = nc.dram_tensor("dump_g", [128, NB * 2 * HC], BF16,
                                    kind="ExternalOutput")
            dump_aot = nc.dram_tensor("dump_aot", [128, NB * 2 * HC], BF16,
                                      kind="ExternalOutput")
            tc.strict_bb_all_engine_barrier()
            nc.sync.dma_start(dump_hj[:], hj_table[:])
            nc.sync.dma_start(dump_ao[:], ao_table[:])
            nc.sync.dma_start(dump_sd[:], stage_d[:])
            nc.sync.dma_start(dump_g[:], last_G[:].rearrange("p a b -> p (a b)"))
            nc.sync.dma_start(dump_aot[:], last_AO[:].rearrange("p a b -> p (a b)"))

    nc.compile()
    return nc


def kernel(**inputs):
    cfg = _cfg(N_NODES, N_EDGES)
    in_maps, meta = _prep(inputs, cfg)
    nc = _build_program(cfg, meta["K"], meta["KL"], meta["KH"])

    from concourse import bass_utils
    res = bass_utils.run_bass_kernel_spmd(
        nc, in_maps, core_ids=list(range(cfg["NC"])),
        trace=bool(int(os.environ.get("GAT_TRACE", "0"))),
    )
    kernel.last_result = res  # stash for test harness (exec_time_ns etc.)
    kernel.last_ctx = (nc, in_maps, cfg)

    NSHARD = cfg["NSHARD"]
    out_full = np.zeros((cfg["NC"] * NSHARD, HC), dtype=np.float32)
    for c in range(cfg["NC"]):
        out_full[c * NSHARD:(c + 1) * NSHARD] = res.results[c]["out"]
    return out_full[:cfg["N"]]



# revision 3
# speedup vs baseline: 74.8473x; 74.8473x over previous
"""Bass/Trainium2 kernel for nn_BitGatConv (GAT-style message passing), v2.

Self-contained: takes full inputs, shards edges by destination window across
8 NeuronCores (SPMD, one program), returns the full [N, HC] output.

v2 changes vs the original baseline:
  - Balanced window packing: nodes are permuted so every 64-node destination
    window has <= K*128 in-edges with K=8 uniform; bin padding drops from
    ~25% to ~2.5%.
  - Overlapped A/B gather views: the int16 index-reach split is handled by
    two OVERLAPPING row views of one hj table ([0, 32768) and
    [N_PAD-32768, N_PAD)); edges with sources in the overlap are assigned to
    whichever half has room, so each window uses exactly 4 A-bins + 4 B-bins.
  - The att_i edge gather (previously 32MB/core of 256B-descriptor DMA) is
    eliminated: per-bin one-hot matrices are generated on device (iota
    is_equal) for the scatter, and att_i[tgt] is computed per bin as a
    64-contraction matmul onehotB.T @ att_win on the PE.

Algorithm (per core, rotated node ids so all cores run the same program):
  Phase A (build): hj = nodes_ft @ [W | W@A2] -> bf16 table [N_PAD, 128] in
    DRAM; att_i windows = nodes_ft @ (W@A1) for the core's own shard kept
    resident in SBUF as [64, NPC, 64] bf16.
  Phase B (edges): per group of 4 windows (32 bins of 128 edge slots):
    gather hj rows by src (two dma_gather calls, A/B views); generate
    onehotA [128e, 64w] via is_equal(tgt_slot, iota); att_i per bin via
    matmul(onehotB [64w,128e], att_win); s = att_i + att_j;
    l = leaky_relu(s); x = exp(l); payload = [x*h | x]; scatter-accumulate
    via matmul(onehotA, payload) into per-window-pair PSUM.
  Phase C (flush): out = numer / (denom + 1e-16) + bias.
  No segment-max subtraction: logits are bounded (~|s|<12) so exp is safe,
  and softmax is shift-free identical.
"""

import math
import os
import sys
from contextlib import ExitStack

import numpy as np

for _p in ("/opt/trn_rl_repo",):
    if _p not in sys.path:
        sys.path.insert(0, _p)

import ml_dtypes  # noqa: E402

BF16_NP = ml_dtypes.bfloat16

# ---------------------------------------------------------------------------
# Problem constants (hardcoded per contest rules)
N_NODES = 50000
N_EDGES = 800000
IN_CH = 128
HC = 64
NEG_SLOPE = 0.2
N_CORES = 8
W_WIN = 64          # nodes per destination window
K_BINS = 8          # bins (of 128 edge slots) per window
JA = 4              # bins gathered from view A (rest from view B)
VIEW = 32768        # rows per gather view (int16 index reach)


def _pack_windows(tgt_global, n_nodes, nw):
    """Balanced-assignment of nodes to nw windows of <=64 nodes, minimizing
    the max window in-degree sum. Zig-zag LPT: process nodes in descending-
    degree chunks of nw, pairing heaviest nodes with lightest windows.
    Returns perm: node -> global slot id (window*64 + position)."""
    deg = np.bincount(tgt_global, minlength=n_nodes).astype(np.int64)
    order = np.argsort(-deg, kind="stable")
    weights = np.zeros(nw, dtype=np.int64)
    counts = np.zeros(nw, dtype=np.int64)
    win_of_node = np.empty(n_nodes, dtype=np.int64)
    pos = 0
    while pos < n_nodes:
        chunk = order[pos:pos + nw]
        worder = np.argsort(weights, kind="stable")  # lightest first
        wsel = worder[: len(chunk)]
        win_of_node[chunk] = wsel
        np.add.at(weights, wsel, deg[chunk])
        np.add.at(counts, wsel, 1)
        pos += nw
    assert counts.max() <= W_WIN
    # slot within window by arrival order
    slot_in_win = np.zeros(n_nodes, dtype=np.int64)
    occupied = np.zeros(nw, dtype=np.int64)
    for nd in order:  # deterministic fill
        w = win_of_node[nd]
        slot_in_win[nd] = occupied[w]
        occupied[w] += 1
    perm = win_of_node * W_WIN + slot_in_win
    return perm, int(weights.max())


def _cfg(npc):
    nw = N_CORES * npc
    n_pad = nw * W_WIN
    nshard = npc * W_WIN
    gw = 4 if npc % 4 == 0 else 3
    assert npc % gw == 0 and npc % 2 == 0
    return dict(
        N=N_NODES, E=N_EDGES, NC=N_CORES, W=W_WIN, NPC=npc, NW=nw,
        N_PAD=n_pad, NSHARD=nshard, GROUP_NW=gw, K=K_BINS, JA=JA,
        B=npc * K_BINS, NB=gw * K_BINS, NGROUPS=npc // gw,
        NPAIR=npc // 2, BOFF=n_pad - VIEW,
        T_TILES=n_pad // 128, SHARD_TILES=nshard // 128,
    )


def _choose_cfg(tgt_global):
    for npc in (100, 104, 108):
        cfg = _cfg(npc)
        perm, maxw = _pack_windows(tgt_global, N_NODES, cfg["NW"])
        if maxw <= K_BINS * 128:
            cfg["perm"] = perm
            return cfg
    raise AssertionError(f"window packing failed (maxw={maxw})")


def _wrap16(stream2d):
    # [NC, L] -> [NC, 128, L//16] in dma_gather wrapped layout
    # (idx i -> [i%16, i//16], replicated 8x across partition groups)
    ncc, L = stream2d.shape
    w = stream2d.reshape(ncc, L // 16, 16).transpose(0, 2, 1)
    return np.ascontiguousarray(np.tile(w, (1, 8, 1)))


def _prep(inputs, cfg):
    """Host-side preprocessing: pack + shard + index/stream building."""
    NC, W, NPC = cfg["NC"], cfg["W"], cfg["NPC"]
    N_PAD, NSHARD, BOFF = cfg["N_PAD"], cfg["NSHARD"], cfg["BOFF"]
    K, GW, B, NB = cfg["K"], cfg["GROUP_NW"], cfg["B"], cfg["NB"]
    NG = cfg["NGROUPS"]

    nodes_ft = np.asarray(inputs["nodes_ft"], dtype=np.float32)
    adj = np.asarray(inputs["adj_list"])
    weight = np.asarray(inputs["weight"], dtype=np.float32)
    a1 = np.asarray(inputs["att_layer_1"], dtype=np.float32)
    a2 = np.asarray(inputs["att_layer_2"], dtype=np.float32)
    bias = np.asarray(inputs["bias"], dtype=np.float32)

    tgt = adj[0].astype(np.int64)
    src = adj[1].astype(np.int64)
    perm = cfg["perm"]

    tslot = perm[tgt]                      # global dst slot
    sslot = perm[src]                      # global src slot
    core = tslot // NSHARD
    lw = (tslot % NSHARD) // W             # local window 0..NPC-1
    wslot = tslot % W                      # slot within window 0..63
    srot = (sslot - core * NSHARD) % N_PAD

    # class: 0 = A-forced (srot < BOFF), 1 = free, 2 = B-forced (>= VIEW)
    cls = np.where(srot < BOFF, 0, np.where(srot < VIEW, 1, 2))

    wkey = core * NPC + lw                 # global window id (core-major)
    NWIN = NC * NPC
    order = np.argsort(wkey * 4 + cls, kind="stable")
    eo_w = wkey[order]
    cnt = np.bincount(wkey, minlength=NWIN)
    cntA = np.bincount(wkey[cls == 0], minlength=NWIN)
    starts = np.zeros(NWIN + 1, dtype=np.int64)
    starts[1:] = np.cumsum(cnt)
    CAP = JA * 128
    assert cnt.max() <= K * 128, cnt.max()
    assert cntA.max() <= CAP, cntA.max()
    assert np.bincount(wkey[cls == 2], minlength=NWIN).max() <= CAP
    nA = cntA + np.maximum(0, cnt - CAP - cntA)   # A-set size per window
    assert (nA <= CAP).all() and (cnt - nA <= CAP).all()

    r = np.arange(cfg["E"], dtype=np.int64) - starts[eo_w]
    inA = r < nA[eo_w]
    rnk = np.where(inA, r, r - nA[eo_w])
    j = rnk // 128                          # bin index within half (0..3)
    p = rnk % 128
    assert j.max() < JA

    c_e = core[order]
    lw_e = lw[order]
    g_e = lw_e // GW
    wl_e = lw_e % GW
    # bin column within group (G-tile order): A bins 0..GW*JA-1, then B bins
    colg = np.where(inA, wl_e * JA + j, GW * JA + wl_e * JA + j)
    b_core = g_e * NB + colg                # bin id within core 0..B-1

    # gather index streams (per view), flat per core
    LH = NG * (GW * JA) * 128               # idx per core per view
    hjA = np.zeros((NC, LH), dtype=np.int16)
    hjB = np.zeros((NC, LH), dtype=np.int16)
    iposA = (g_e * (GW * JA) + wl_e * JA + j) * 128 + p
    mA = inA
    hjA[c_e[mA], iposA[mA]] = srot[order][mA].astype(np.int16)
    mB = ~inA
    hjB[c_e[mB], iposA[mB]] = (srot[order][mB] - BOFF).astype(np.int16)

    # tgt slot stream [NC, 128, B] bf16 (64.0 = pad -> zero one-hot column)
    tgtA = np.full((NC, 128, B), np.float32(W), dtype=np.float32)
    tgtA[c_e, p, b_core] = wslot[order].astype(np.float32)

    # one-hot B stream [NC, 64, B*128] bf16
    ohB = np.zeros((NC, W, B * 128), dtype=np.float32)
    ohB[c_e, wslot[order], b_core * 128 + p] = 1.0

    # constants
    whwj = np.concatenate([weight, weight @ a2], axis=1).astype(BF16_NP)
    wi = (weight @ a1).astype(BF16_NP)
    iota64 = np.tile(np.arange(W, dtype=np.float32), (128, 1)).astype(BF16_NP)
    NPAIR = cfg["NPAIR"]
    bias_full = np.tile(bias[None, :], (128, NPAIR)).astype(np.float32)

    # permuted, transposed, padded node features (bf16)
    base = np.zeros((IN_CH, N_PAD), dtype=np.float32)
    base[:, perm] = nodes_ft.T

    in_maps = []
    for c in range(NC):
        nftT = np.ascontiguousarray(np.roll(base, -c * NSHARD, axis=1))
        in_maps.append({
            "nodes_ftT": nftT.astype(BF16_NP),
            "whwj": whwj, "wi": wi,
            "iota64": iota64,
            "hjA_idx": _wrap16(hjA[c:c + 1])[0],
            "hjB_idx": _wrap16(hjB[c:c + 1])[0],
            "tgtA": tgtA[c].astype(BF16_NP),
            "onehotB": ohB[c].astype(BF16_NP),
            "bias_bc": bias_full,
        })
    return in_maps


def _build_program(cfg, debug_dump=False, phase_limit="full", repeat=1):
    import concourse.bacc as bacc
    import concourse.bass as bass
    import concourse.mybir as mybir
    import concourse.tile as tile
    from concourse.bass import broadcast_tensor_aps

    BF16 = mybir.dt.bfloat16
    F32 = mybir.dt.float32
    I16 = mybir.dt.int16
    ALU = mybir.AluOpType
    ACT = mybir.ActivationFunctionType

    NPC, N_PAD, NSHARD = cfg["NPC"], cfg["N_PAD"], cfg["NSHARD"]
    T_TILES, SHARD_TILES = cfg["T_TILES"], cfg["SHARD_TILES"]
    GW, K, B, NB, NG = (cfg["GROUP_NW"], cfg["K"], cfg["B"], cfg["NB"],
                        cfg["NGROUPS"])
    NPAIR, BOFF = cfg["NPAIR"], cfg["BOFF"]
    NBA = GW * JA                      # A bins per group
    LH = NG * NBA * 128                # gather idx per core per view

    nc = bacc.Bacc("TRN2", target_bir_lowering=False, debug=False,
                   num_swdge_queues=4)

    nodes_ftT = nc.dram_tensor("nodes_ftT", [IN_CH, N_PAD], BF16, kind="ExternalInput")
    whwj_d = nc.dram_tensor("whwj", [IN_CH, 2 * HC], BF16, kind="ExternalInput")
    wi_d = nc.dram_tensor("wi", [IN_CH, HC], BF16, kind="ExternalInput")
    iota_d = nc.dram_tensor("iota64", [128, W_WIN], BF16, kind="ExternalInput")
    hjA_d = nc.dram_tensor("hjA_idx", [128, LH // 16], I16, kind="ExternalInput")
    hjB_d = nc.dram_tensor("hjB_idx", [128, LH // 16], I16, kind="ExternalInput")
    tgtA_d = nc.dram_tensor("tgtA", [128, B], BF16, kind="ExternalInput")
    ohB_d = nc.dram_tensor("onehotB", [W_WIN, B * 128], BF16, kind="ExternalInput")
    bias_d = nc.dram_tensor("bias_bc", [128, NPAIR * HC], F32, kind="ExternalInput")
    out_d = nc.dram_tensor("out", [NSHARD, HC], F32, kind="ExternalOutput")

    hj_table = nc.dram_tensor("hj_table", [N_PAD, 2 * HC], BF16, kind="Internal")

    do_build = phase_limit != "noop"
    do_gather = phase_limit in ("gather", "nomm", "full")
    do_dve = phase_limit in ("nomm", "full")
    do_mm = phase_limit == "full"

    with tile.TileContext(nc) as tc, ExitStack() as ctx:
        const_pool = ctx.enter_context(tc.tile_pool(name="const", bufs=1))
        b_in = ctx.enter_context(tc.tile_pool(name="b_in", bufs=4))
        b_ps = ctx.enter_context(tc.tile_pool(name="b_ps", bufs=2, space="PSUM"))
        aw_ps = ctx.enter_context(tc.tile_pool(name="aw_ps", bufs=1, space="PSUM"))
        b_st = ctx.enter_context(tc.tile_pool(name="b_st", bufs=4))
        idx_pool = ctx.enter_context(tc.tile_pool(name="idx", bufs=4))
        g_pool = ctx.enter_context(tc.tile_pool(name="gp", bufs=2))
        ohb_pool = ctx.enter_context(tc.tile_pool(name="ohb", bufs=2))
        oha_pool = ctx.enter_context(tc.tile_pool(name="oha", bufs=2))
        s_pool = ctx.enter_context(tc.tile_pool(name="sp", bufs=2))
        atti_ps = ctx.enter_context(tc.tile_pool(name="attips", bufs=1, space="PSUM"))
        mm_ps = ctx.enter_context(tc.tile_pool(name="mmps", bufs=2, space="PSUM"))
        fl_pool = ctx.enter_context(tc.tile_pool(name="fl", bufs=1))

        whwj_sb = const_pool.tile([IN_CH, 2 * HC], BF16)
        nc.sync.dma_start(whwj_sb[:], whwj_d[:])
        wi_sb = const_pool.tile([IN_CH, HC], BF16)
        nc.sync.dma_start(wi_sb[:], wi_d[:])
        iota_sb = const_pool.tile([128, W_WIN], BF16)
        nc.sync.dma_start(iota_sb[:], iota_d[:])
        tgtA_sb = const_pool.tile([128, B], BF16)
        nc.sync.dma_start(tgtA_sb[:], tgtA_d[:])
        bias_sb = const_pool.tile([128, NPAIR * HC], F32)
        nc.sync.dma_start(bias_sb[:], bias_d[:])
        attiwin = const_pool.tile([W_WIN, NPC, HC], BF16)

        def emit_once(rep):
            # ---- Phase A: build hj table (DRAM) + att_i windows (SBUF)
            # Each iteration handles 256 nodes. The two matmuls split nodes
            # even/odd so store-partition p holds nodes {2p, 2p+1}: the table
            # write is then 512B-contiguous per partition (full DMA rate;
            # 256B-row writes would eat the small-descriptor RMW penalty).
            for q in range(T_TILES // 2 if do_build else 0):
                dmae = nc.sync if q % 2 == 0 else nc.scalar
                nf = b_in.tile([128, 256], BF16, name="nf")
                dmae.dma_start(nf[:], nodes_ftT[:, 256 * q:256 * (q + 1)])
                nfv = nf[:].rearrange("p (n two) -> p two n", two=2)
                ps = b_ps.tile([128, 2, 2 * HC], F32, name="bps")
                for u in range(2):
                    nc.tensor.matmul(ps[:, u, :], nfv[:, u, :], whwj_sb[:],
                                     start=True, stop=True)
                st = b_st.tile([128, 2, 2 * HC], BF16, name="bst")
                if q % 2 == 0:
                    nc.vector.tensor_copy(st[:], ps[:])
                else:
                    nc.scalar.copy(st[:], ps[:])
                dmae.dma_start(
                    hj_table[256 * q:256 * (q + 1), :].rearrange(
                        "(p two) b -> p two b", p=128),
                    st[:].rearrange("p two b -> p two b"))
                if 2 * q < SHARD_TILES:
                    # att_i for the core's own shard -> resident SBUF windows
                    for wv in range(4):
                        aps = aw_ps.tile([W_WIN, HC], F32, tag=f"aw{wv % 2}",
                                         name="aw")
                        nc.tensor.matmul(
                            aps[:], nf[:, 64 * wv:64 * (wv + 1)],
                            wi_sb[:], start=True, stop=True)
                        if wv % 2 == 0:
                            nc.vector.tensor_copy(
                                attiwin[:, 4 * q + wv, :], aps[:])
                        else:
                            nc.scalar.copy(
                                attiwin[:, 4 * q + wv, :], aps[:])

            # ---- Phase B: edge processing
            stage_n = fl_pool.tile([128, NPAIR * HC], F32, tag="sn", name="sn")
            stage_d = fl_pool.tile([128, NPAIR * HC], F32, tag="sd", name="sd")

            def col_to_wj(col):
                if col < NBA:
                    return col // JA, col % JA
                c2 = col - NBA
                return c2 // JA, JA + c2 % JA

            last_G = last_ohA = None
            for g in range(NG if do_gather else 0):
                sl = idx_pool.tile([128, NBA * 8], I16, tag="sl", name="sl")
                nc.sync.dma_start(sl[:], hjA_d[:, g * NBA * 8:(g + 1) * NBA * 8])
                sh = idx_pool.tile([128, NBA * 8], I16, tag="sh", name="sh")
                nc.sync.dma_start(sh[:], hjB_d[:, g * NBA * 8:(g + 1) * NBA * 8])
                ohB = ohb_pool.tile([W_WIN, NB * 128], BF16, tag="ohB",
                                    name="ohB")
                # all Phase B DMA issue stays on the sync queue: the scalar
                # (ACT) queue runs exp, and a dma_start queued behind exp
                # would stall the next group's gather pipeline.
                nc.sync.dma_start(ohB[:], ohB_d[:, g * NB * 128:(g + 1) * NB * 128])

                G = g_pool.tile([128, NB, 2 * HC], BF16, tag="G", name="G")
                sp = bool(int(os.environ.get("GAT_SINGLE_PACKET", "0")))
                for half, idxs, lo in ((0, sl, 0), (1, sh, BOFF)):
                    nc.gpsimd.dma_gather(
                        out_ap=G[:, half * NBA:(half + 1) * NBA, :],
                        in_ap=hj_table[lo:lo + VIEW, :],
                        idxs_ap=idxs[:],
                        num_idxs=NBA * 128,
                        num_idxs_reg=NBA * 128,
                        elem_size=2 * HC, queue_num=(2 * g + half) % 4,
                        single_packet=sp,
                    )
                last_G = G

                if not do_dve:
                    continue
                # one-hot A [128e, col, 64w] via is_equal(tgt_slot, iota)
                ohA = oha_pool.tile([128, NB, W_WIN], BF16, tag="ohA",
                                    name="ohA")
                t3 = tgtA_sb[:, g * NB:(g + 1) * NB].rearrange(
                    "p (b one) -> p b one", one=1)
                i3 = iota_sb[:].rearrange("p (one w) -> p one w", one=1)
                a_bc, b_bc = broadcast_tensor_aps(t3, i3)
                nc.vector.tensor_tensor(out=ohA[:], in0=a_bc, in1=b_bc,
                                        op=ALU.is_equal)
                last_ohA = ohA

                # att_i per bin: onehotB.T @ att_win  (64-contraction)
                S = s_pool.tile([128, NB, HC], BF16, tag="S", name="S")
                NBH = NB // 2
                for hf in range(2 if do_mm else 0):
                    apsg = atti_ps.tile([128, NBH * HC], F32, tag="atti",
                                        name="atti")
                    for c2 in range(NBH):
                        col = hf * NBH + c2
                        w, _j = col_to_wj(col)
                        lwin = g * GW + w
                        nc.tensor.matmul(
                            apsg[:, c2 * HC:(c2 + 1) * HC],
                            ohB[:, col * 128:(col + 1) * 128],
                            attiwin[:, lwin, :],
                            start=True, stop=True, skip_group_check=True,
                        )
                    # s = att_j + att_i
                    nc.vector.tensor_tensor(
                        out=S[:, hf * NBH:(hf + 1) * NBH, :],
                        in0=G[:, hf * NBH:(hf + 1) * NBH, HC:2 * HC],
                        in1=apsg[:].rearrange("p (b c) -> p b c", c=HC),
                        op=ALU.add)
                if not do_mm:
                    nc.vector.tensor_copy(S[:], G[:, :, HC:2 * HC])
                # l = max(0.2*s, s)  (leaky relu)
                nc.vector.scalar_tensor_tensor(
                    out=S[:], in0=S[:], scalar=NEG_SLOPE, in1=S[:],
                    op0=ALU.mult, op1=ALU.max)
                # x = exp(l) -> overwrite att_j half of G
                nc.scalar.activation(G[:, :, HC:2 * HC], S[:], ACT.Exp)
                # y = h * x -> overwrite h half of G
                nc.vector.tensor_tensor(
                    out=G[:, :, 0:HC], in0=G[:, :, 0:HC],
                    in1=G[:, :, HC:2 * HC], op=ALU.mult)

                pair_tiles = {}
                for col in range(NB if do_mm else 0):
                    w, jj = col_to_wj(col)
                    lwin = g * GW + w
                    pr, par = lwin // 2, lwin % 2
                    if jj == 0 and par == 0:
                        pair_tiles[pr] = mm_ps.tile(
                            [128, 2 * HC], F32, tag="pp", name=f"pp{pr % 4}")
                    ps_t = pair_tiles[pr]
                    nc.tensor.matmul(
                        ps_t[HC * par:HC * par + HC, :],
                        ohA[:, col, :],
                        G[:, col, :],
                        start=(jj == 0), stop=(jj == K - 1),
                        tile_position=(0, HC * par),
                        skip_group_check=True,
                    )
                    if jj == K - 1 and par == 1:
                        nc.vector.tensor_copy(
                            stage_n[:, HC * pr:HC * (pr + 1)], ps_t[:, 0:HC])
                        nc.vector.tensor_copy(
                            stage_d[:, HC * pr:HC * (pr + 1)],
                            ps_t[:, HC:2 * HC])
                        del pair_tiles[pr]

            # ---- Phase C: out = numer / (denom + eps) + bias
            if not do_mm:
                nc.vector.memset(stage_n[:], 0.0)
                nc.vector.memset(stage_d[:], 1.0)
            nc.vector.tensor_scalar_add(stage_d[:], stage_d[:], 1e-16)
            lnd = fl_pool.tile([128, NPAIR * HC], F32, tag="lnd", name="lnd")
            nc.scalar.activation(lnd[:], stage_d[:], ACT.Ln)
            nc.scalar.activation(lnd[:], lnd[:], ACT.Exp, scale=-1.0)
            nc.vector.tensor_tensor(out=stage_n[:], in0=stage_n[:], in1=lnd[:],
                                    op=ALU.mult)
            nc.vector.tensor_tensor(out=stage_n[:], in0=stage_n[:],
                                    in1=bias_sb[:], op=ALU.add)

            out_view = out_d[:].rearrange("(pr p) c -> p pr c", p=128)
            st_view = stage_n[:].rearrange("p (pr c) -> p pr c", c=HC)
            nc.sync.dma_start(out_view, st_view)
            return last_G, last_ohA, stage_d

        for rep in range(repeat):
            last_G, last_ohA, stage_d = emit_once(rep)
            if repeat > 1:
                tc.strict_bb_all_engine_barrier()

        if debug_dump:
            dump_hj = nc.dram_tensor("dump_hj", [N_PAD, 2 * HC], BF16,
                                     kind="ExternalOutput")
            dump_aw = nc.dram_tensor("dump_aw", [W_WIN, NPC * HC], BF16,
                                     kind="ExternalOutput")
            dump_sd = nc.dram_tensor("dump_sd", [128, NPAIR * HC], F32,
                                     kind="ExternalOutput")
            dump_g = nc.dram_tensor("dump_g", [128, NB * 2 * HC], BF16,
                                    kind="ExternalOutput")
            dump_oha = nc.dram_tensor("dump_oha", [128, NB * W_WIN], BF16,
                                      kind="ExternalOutput")
            tc.strict_bb_all_engine_barrier()
            nc.sync.dma_start(dump_hj[:], hj_table[:])
            nc.sync.dma_start(dump_aw[:],
                              attiwin[:].rearrange("p a b -> p (a b)"))
            nc.sync.dma_start(dump_sd[:], stage_d[:])
            nc.sync.dma_start(dump_g[:], last_G[:].rearrange("p a b -> p (a b)"))
            nc.sync.dma_start(dump_oha[:],
                              last_ohA[:].rearrange("p a b -> p (a b)"))

    nc.compile()
    return nc


def kernel(**inputs):
    adj = np.asarray(inputs["adj_list"])
    cfg = _choose_cfg(adj[0].astype(np.int64))
    in_maps = _prep(inputs, cfg)
    nc = _build_program(cfg)

    from concourse import bass_utils
    res = bass_utils.run_bass_kernel_spmd(
        nc, in_maps, core_ids=list(range(cfg["NC"])),
        trace=bool(int(os.environ.get("GAT_TRACE", "0"))),
    )
    kernel.last_result = res  # stash for test harness (exec_time_ns etc.)
    kernel.last_ctx = (nc, in_maps, cfg)

    NSHARD = cfg["NSHARD"]
    out_slots = np.zeros((cfg["N_PAD"], HC), dtype=np.float32)
    for c in range(cfg["NC"]):
        out_slots[c * NSHARD:(c + 1) * NSHARD] = res.results[c]["out"]
    return out_slots[cfg["perm"]]


# revision 4
# speedup vs baseline: 91.6643x; 1.2247x over previous
"""Bass/Trainium2 kernel for nn_BitGatConv (GAT-style message passing), v2.

Self-contained: takes full inputs, shards edges by destination window across
8 NeuronCores (SPMD, one program), returns the full [N, HC] output.

v2 changes vs the original baseline:
  - Balanced window packing: nodes are permuted so every 64-node destination
    window has <= K*128 in-edges with K=8 uniform; bin padding drops from
    ~25% to ~2.5%.
  - Overlapped A/B gather views: the int16 index-reach split is handled by
    two OVERLAPPING row views of one hj table ([0, 32768) and
    [N_PAD-32768, N_PAD)); edges with sources in the overlap are assigned to
    whichever half has room, so each window uses exactly 4 A-bins + 4 B-bins.
  - The att_i edge gather (previously 32MB/core of 256B-descriptor DMA) is
    eliminated: per-bin one-hot matrices are generated on device (iota
    is_equal) for the scatter, and att_i[tgt] is computed per bin as a
    64-contraction matmul onehotB.T @ att_win on the PE.

Algorithm (per core, rotated node ids so all cores run the same program):
  Phase A (build): hj = nodes_ft @ [W | W@A2] -> bf16 table [N_PAD, 128] in
    DRAM; att_i windows = nodes_ft @ (W@A1) for the core's own shard kept
    resident in SBUF as [64, NPC, 64] bf16.
  Phase B (edges): per group of 4 windows (32 bins of 128 edge slots):
    gather hj rows by src (two dma_gather calls, A/B views); generate
    onehotA [128e, 64w] via is_equal(tgt_slot, iota); att_i per bin via
    matmul(onehotB [64w,128e], att_win); s = att_i + att_j;
    l = leaky_relu(s); x = exp(l); payload = [x*h | x]; scatter-accumulate
    via matmul(onehotA, payload) into per-window-pair PSUM.
  Phase C (flush): out = numer / (denom + 1e-16) + bias.
  No segment-max subtraction: logits are bounded (~|s|<12) so exp is safe,
  and softmax is shift-free identical.
"""

import math
import os
import sys
from contextlib import ExitStack

import numpy as np

for _p in ("/opt/trn_rl_repo",):
    if _p not in sys.path:
        sys.path.insert(0, _p)

import ml_dtypes  # noqa: E402

BF16_NP = ml_dtypes.bfloat16

# ---------------------------------------------------------------------------
# Problem constants (hardcoded per contest rules)
N_NODES = 50000
N_EDGES = 800000
IN_CH = 128
HC = 64
NEG_SLOPE = 0.2
N_CORES = 8
W_WIN = 64          # nodes per destination window
K_BINS = 8          # bins (of 128 edge slots) per window
JA = 4              # bins gathered from view A (rest from view B)
VIEW = 32768        # rows per gather view (int16 index reach)


def _pack_windows(tgt_global, n_nodes, nw):
    """Balanced-assignment of nodes to nw windows of <=64 nodes, minimizing
    the max window in-degree sum. Zig-zag LPT: process nodes in descending-
    degree chunks of nw, pairing heaviest nodes with lightest windows.
    Returns perm: node -> global slot id (window*64 + position)."""
    deg = np.bincount(tgt_global, minlength=n_nodes).astype(np.int64)
    order = np.argsort(-deg, kind="stable")
    weights = np.zeros(nw, dtype=np.int64)
    counts = np.zeros(nw, dtype=np.int64)
    win_of_node = np.empty(n_nodes, dtype=np.int64)
    pos = 0
    while pos < n_nodes:
        chunk = order[pos:pos + nw]
        worder = np.argsort(weights, kind="stable")  # lightest first
        wsel = worder[: len(chunk)]
        win_of_node[chunk] = wsel
        np.add.at(weights, wsel, deg[chunk])
        np.add.at(counts, wsel, 1)
        pos += nw
    assert counts.max() <= W_WIN
    # slot within window by arrival order
    slot_in_win = np.zeros(n_nodes, dtype=np.int64)
    occupied = np.zeros(nw, dtype=np.int64)
    for nd in order:  # deterministic fill
        w = win_of_node[nd]
        slot_in_win[nd] = occupied[w]
        occupied[w] += 1
    perm = win_of_node * W_WIN + slot_in_win
    return perm, int(weights.max())


def _cfg(npc):
    nw = N_CORES * npc
    n_pad = nw * W_WIN
    nshard = npc * W_WIN
    gw = 4 if npc % 4 == 0 else 3
    assert npc % gw == 0 and npc % 2 == 0
    return dict(
        N=N_NODES, E=N_EDGES, NC=N_CORES, W=W_WIN, NPC=npc, NW=nw,
        N_PAD=n_pad, NSHARD=nshard, GROUP_NW=gw, K=K_BINS, JA=JA,
        B=npc * K_BINS, NB=gw * K_BINS, NGROUPS=npc // gw,
        NPAIR=npc // 2, BOFF=n_pad - VIEW,
        T_TILES=n_pad // 128, SHARD_TILES=nshard // 128,
    )


def _choose_cfg(tgt_global):
    for npc in (100, 104, 108):
        cfg = _cfg(npc)
        perm, maxw = _pack_windows(tgt_global, N_NODES, cfg["NW"])
        if maxw <= K_BINS * 128:
            cfg["perm"] = perm
            return cfg
    raise AssertionError(f"window packing failed (maxw={maxw})")


def _wrap16(stream2d):
    # [NC, L] -> [NC, 128, L//16] in dma_gather wrapped layout
    # (idx i -> [i%16, i//16], replicated 8x across partition groups)
    ncc, L = stream2d.shape
    w = stream2d.reshape(ncc, L // 16, 16).transpose(0, 2, 1)
    return np.ascontiguousarray(np.tile(w, (1, 8, 1)))


def _prep(inputs, cfg):
    """Host-side preprocessing: pack + shard + index/stream building."""
    NC, W, NPC = cfg["NC"], cfg["W"], cfg["NPC"]
    N_PAD, NSHARD, BOFF = cfg["N_PAD"], cfg["NSHARD"], cfg["BOFF"]
    K, GW, B, NB = cfg["K"], cfg["GROUP_NW"], cfg["B"], cfg["NB"]
    NG = cfg["NGROUPS"]

    nodes_ft = np.asarray(inputs["nodes_ft"], dtype=np.float32)
    adj = np.asarray(inputs["adj_list"])
    weight = np.asarray(inputs["weight"], dtype=np.float32)
    a1 = np.asarray(inputs["att_layer_1"], dtype=np.float32)
    a2 = np.asarray(inputs["att_layer_2"], dtype=np.float32)
    bias = np.asarray(inputs["bias"], dtype=np.float32)

    tgt = adj[0].astype(np.int64)
    src = adj[1].astype(np.int64)
    perm = cfg["perm"]

    tslot = perm[tgt]                      # global dst slot
    sslot = perm[src]                      # global src slot
    core = tslot // NSHARD
    lw = (tslot % NSHARD) // W             # local window 0..NPC-1
    wslot = tslot % W                      # slot within window 0..63
    srot = (sslot - core * NSHARD) % N_PAD

    # class: 0 = A-forced (srot < BOFF), 1 = free, 2 = B-forced (>= VIEW)
    cls = np.where(srot < BOFF, 0, np.where(srot < VIEW, 1, 2))

    wkey = core * NPC + lw                 # global window id (core-major)
    NWIN = NC * NPC
    order1 = np.argsort(wkey * 4 + cls, kind="stable")
    eo_w = wkey[order1]
    cnt = np.bincount(wkey, minlength=NWIN)
    cntA = np.bincount(wkey[cls == 0], minlength=NWIN)
    starts = np.zeros(NWIN + 1, dtype=np.int64)
    starts[1:] = np.cumsum(cnt)
    CAP = JA * 128
    assert cnt.max() <= K * 128, cnt.max()
    assert cntA.max() <= CAP, cntA.max()
    assert np.bincount(wkey[cls == 2], minlength=NWIN).max() <= CAP
    nA = cntA + np.maximum(0, cnt - CAP - cntA)   # A-set size per window
    assert (nA <= CAP).all() and (cnt - nA <= CAP).all()

    r = np.arange(cfg["E"], dtype=np.int64) - starts[eo_w]
    inA1 = r < nA[eo_w]
    halfid = np.empty(cfg["E"], dtype=np.int64)   # original-edge space
    halfid[order1] = (~inA1).astype(np.int64)     # 0 = A view, 1 = B view

    # final slot order: within (window, half) sort by source row so gather
    # descriptors walk HBM mostly monotonically (row locality)
    order = np.lexsort((srot, halfid, wkey))
    grp2 = wkey[order] * 2 + halfid[order]
    cnt2 = np.bincount(grp2, minlength=NWIN * 2)
    starts2 = np.zeros(NWIN * 2 + 1, dtype=np.int64)
    starts2[1:] = np.cumsum(cnt2)
    rnk = np.arange(cfg["E"], dtype=np.int64) - starts2[grp2]
    inA = halfid[order] == 0
    j = rnk // 128                          # bin index within half (0..3)
    p = rnk % 128
    assert j.max() < JA

    c_e = core[order]
    lw_e = lw[order]
    g_e = lw_e // GW
    wl_e = lw_e % GW
    # bin column within group (G-tile order): A bins 0..GW*JA-1, then B bins
    colg = np.where(inA, wl_e * JA + j, GW * JA + wl_e * JA + j)
    b_core = g_e * NB + colg                # bin id within core 0..B-1

    # gather index streams (per view), flat per core
    LH = NG * (GW * JA) * 128               # idx per core per view
    hjA = np.zeros((NC, LH), dtype=np.int16)
    hjB = np.zeros((NC, LH), dtype=np.int16)
    iposA = (g_e * (GW * JA) + wl_e * JA + j) * 128 + p
    mA = inA
    hjA[c_e[mA], iposA[mA]] = srot[order][mA].astype(np.int16)
    mB = ~inA
    hjB[c_e[mB], iposA[mB]] = (srot[order][mB] - BOFF).astype(np.int16)

    # tgt slot stream [NC, 128, B] bf16 (64.0 = pad -> zero one-hot column)
    tgtA = np.full((NC, 128, B), np.float32(W), dtype=np.float32)
    tgtA[c_e, p, b_core] = wslot[order].astype(np.float32)

    # one-hot B stream [NC, 64, B*128] bf16
    ohB = np.zeros((NC, W, B * 128), dtype=np.float32)
    ohB[c_e, wslot[order], b_core * 128 + p] = 1.0

    # constants
    whwj = np.concatenate([weight, weight @ a2], axis=1).astype(BF16_NP)
    wi = (weight @ a1).astype(BF16_NP)
    iota64 = np.tile(np.arange(W, dtype=np.float32), (128, 1)).astype(BF16_NP)
    NPAIR = cfg["NPAIR"]
    bias_full = np.tile(bias[None, :], (128, NPAIR)).astype(np.float32)

    # permuted, transposed, padded node features (bf16)
    base = np.zeros((IN_CH, N_PAD), dtype=np.float32)
    base[:, perm] = nodes_ft.T

    in_maps = []
    for c in range(NC):
        nftT = np.ascontiguousarray(np.roll(base, -c * NSHARD, axis=1))
        in_maps.append({
            "nodes_ftT": nftT.astype(BF16_NP),
            "whwj": whwj, "wi": wi,
            "iota64": iota64,
            "hjA_idx": _wrap16(hjA[c:c + 1])[0],
            "hjB_idx": _wrap16(hjB[c:c + 1])[0],
            "tgtA": tgtA[c].astype(BF16_NP),
            "onehotB": ohB[c].astype(BF16_NP),
            "bias_bc": bias_full,
        })
    return in_maps


def _build_program(cfg, debug_dump=False, phase_limit="full", repeat=1,
                   single_packet=None):
    import concourse.bacc as bacc
    import concourse.bass as bass
    import concourse.mybir as mybir
    import concourse.tile as tile
    from concourse.bass import broadcast_tensor_aps

    BF16 = mybir.dt.bfloat16
    F32 = mybir.dt.float32
    I16 = mybir.dt.int16
    ALU = mybir.AluOpType
    ACT = mybir.ActivationFunctionType

    NPC, N_PAD, NSHARD = cfg["NPC"], cfg["N_PAD"], cfg["NSHARD"]
    T_TILES, SHARD_TILES = cfg["T_TILES"], cfg["SHARD_TILES"]
    GW, K, B, NB, NG = (cfg["GROUP_NW"], cfg["K"], cfg["B"], cfg["NB"],
                        cfg["NGROUPS"])
    NPAIR, BOFF = cfg["NPAIR"], cfg["BOFF"]
    NBA = GW * JA                      # A bins per group
    LH = NG * NBA * 128                # gather idx per core per view

    nc = bacc.Bacc("TRN2", target_bir_lowering=False, debug=False,
                   num_swdge_queues=4)

    nodes_ftT = nc.dram_tensor("nodes_ftT", [IN_CH, N_PAD], BF16, kind="ExternalInput")
    whwj_d = nc.dram_tensor("whwj", [IN_CH, 2 * HC], BF16, kind="ExternalInput")
    wi_d = nc.dram_tensor("wi", [IN_CH, HC], BF16, kind="ExternalInput")
    iota_d = nc.dram_tensor("iota64", [128, W_WIN], BF16, kind="ExternalInput")
    hjA_d = nc.dram_tensor("hjA_idx", [128, LH // 16], I16, kind="ExternalInput")
    hjB_d = nc.dram_tensor("hjB_idx", [128, LH // 16], I16, kind="ExternalInput")
    tgtA_d = nc.dram_tensor("tgtA", [128, B], BF16, kind="ExternalInput")
    ohB_d = nc.dram_tensor("onehotB", [W_WIN, B * 128], BF16, kind="ExternalInput")
    bias_d = nc.dram_tensor("bias_bc", [128, NPAIR * HC], F32, kind="ExternalInput")
    out_d = nc.dram_tensor("out", [NSHARD, HC], F32, kind="ExternalOutput")

    hj_table = nc.dram_tensor("hj_table", [N_PAD, 2 * HC], BF16, kind="Internal")

    do_build = phase_limit != "noop"
    do_gather = phase_limit in ("gather", "nomm", "full")
    do_dve = phase_limit in ("nomm", "full")
    do_mm = phase_limit == "full"

    with tile.TileContext(nc) as tc, ExitStack() as ctx:
        const_pool = ctx.enter_context(tc.tile_pool(name="const", bufs=1))
        b_in = ctx.enter_context(tc.tile_pool(name="b_in", bufs=4))
        b_ps = ctx.enter_context(tc.tile_pool(name="b_ps", bufs=2, space="PSUM"))
        aw_ps = ctx.enter_context(tc.tile_pool(name="aw_ps", bufs=1, space="PSUM"))
        b_st = ctx.enter_context(tc.tile_pool(name="b_st", bufs=4))
        idx_pool = ctx.enter_context(tc.tile_pool(name="idx", bufs=6))
        g_pool = ctx.enter_context(tc.tile_pool(name="gp", bufs=3))
        ohb_pool = ctx.enter_context(tc.tile_pool(name="ohb", bufs=3))
        oha_pool = ctx.enter_context(tc.tile_pool(name="oha", bufs=2))
        s_pool = ctx.enter_context(tc.tile_pool(name="sp", bufs=2))
        atti_ps = ctx.enter_context(tc.tile_pool(name="attips", bufs=1, space="PSUM"))
        mm_ps = ctx.enter_context(tc.tile_pool(name="mmps", bufs=2, space="PSUM"))
        fl_pool = ctx.enter_context(tc.tile_pool(name="fl", bufs=1))

        whwj_sb = const_pool.tile([IN_CH, 2 * HC], BF16)
        nc.sync.dma_start(whwj_sb[:], whwj_d[:])
        wi_sb = const_pool.tile([IN_CH, HC], BF16)
        nc.sync.dma_start(wi_sb[:], wi_d[:])
        iota_sb = const_pool.tile([128, W_WIN], BF16)
        nc.sync.dma_start(iota_sb[:], iota_d[:])
        tgtA_sb = const_pool.tile([128, B], BF16)
        nc.sync.dma_start(tgtA_sb[:], tgtA_d[:])
        bias_sb = const_pool.tile([128, NPAIR * HC], F32)
        nc.sync.dma_start(bias_sb[:], bias_d[:])
        attiwin = const_pool.tile([W_WIN, NPC, HC], BF16)

        def emit_once(rep):
            # ---- Phase A: build hj table (DRAM) + att_i windows (SBUF)
            # Each iteration handles 256 nodes. The two matmuls split nodes
            # even/odd so store-partition p holds nodes {2p, 2p+1}: the table
            # write is then 512B-contiguous per partition (full DMA rate;
            # 256B-row writes would eat the small-descriptor RMW penalty).
            for q in range(T_TILES // 2 if do_build else 0):
                dmae = nc.sync if q % 2 == 0 else nc.scalar
                nf = b_in.tile([128, 256], BF16, name="nf")
                dmae.dma_start(nf[:], nodes_ftT[:, 256 * q:256 * (q + 1)])
                nfv = nf[:].rearrange("p (n two) -> p two n", two=2)
                ps = b_ps.tile([128, 2, 2 * HC], F32, name="bps")
                for u in range(2):
                    nc.tensor.matmul(ps[:, u, :], nfv[:, u, :], whwj_sb[:],
                                     start=True, stop=True)
                st = b_st.tile([128, 2, 2 * HC], BF16, name="bst")
                if q % 2 == 0:
                    nc.vector.tensor_copy(st[:], ps[:])
                else:
                    nc.scalar.copy(st[:], ps[:])
                dmae.dma_start(
                    hj_table[256 * q:256 * (q + 1), :].rearrange(
                        "(p two) b -> p two b", p=128),
                    st[:].rearrange("p two b -> p two b"))
                if 2 * q < SHARD_TILES:
                    # att_i for the core's own shard -> resident SBUF windows
                    for wv in range(4):
                        aps = aw_ps.tile([W_WIN, HC], F32, tag=f"aw{wv % 2}",
                                         name="aw")
                        nc.tensor.matmul(
                            aps[:], nf[:, 64 * wv:64 * (wv + 1)],
                            wi_sb[:], start=True, stop=True)
                        if wv % 2 == 0:
                            nc.vector.tensor_copy(
                                attiwin[:, 4 * q + wv, :], aps[:])
                        else:
                            nc.scalar.copy(
                                attiwin[:, 4 * q + wv, :], aps[:])

            # ---- Phase B: edge processing
            stage_n = fl_pool.tile([128, NPAIR * HC], F32, tag="sn", name="sn")
            stage_d = fl_pool.tile([128, NPAIR * HC], F32, tag="sd", name="sd")

            def col_to_wj(col):
                if col < NBA:
                    return col // JA, col % JA
                c2 = col - NBA
                return c2 // JA, JA + c2 % JA

            last_G = last_ohA = None
            for g in range(NG if do_gather else 0):
                sl = idx_pool.tile([128, NBA * 8], I16, tag="sl", name="sl")
                nc.sync.dma_start(sl[:], hjA_d[:, g * NBA * 8:(g + 1) * NBA * 8])
                sh = idx_pool.tile([128, NBA * 8], I16, tag="sh", name="sh")
                nc.sync.dma_start(sh[:], hjB_d[:, g * NBA * 8:(g + 1) * NBA * 8])
                ohB = ohb_pool.tile([W_WIN, NB * 128], BF16, tag="ohB",
                                    name="ohB")
                # all Phase B DMA issue stays on the sync queue: the scalar
                # (ACT) queue runs exp, and a dma_start queued behind exp
                # would stall the next group's gather pipeline.
                nc.sync.dma_start(ohB[:], ohB_d[:, g * NB * 128:(g + 1) * NB * 128])

                G = g_pool.tile([128, NB, 2 * HC], BF16, tag="G", name="G")
                sp = (bool(int(os.environ.get("GAT_SINGLE_PACKET", "0")))
                      if single_packet is None else single_packet)
                for half, idxs, lo in ((0, sl, 0), (1, sh, BOFF)):
                    nc.gpsimd.dma_gather(
                        out_ap=G[:, half * NBA:(half + 1) * NBA, :],
                        in_ap=hj_table[lo:lo + VIEW, :],
                        idxs_ap=idxs[:],
                        num_idxs=NBA * 128,
                        num_idxs_reg=NBA * 128,
                        elem_size=2 * HC, queue_num=(2 * g + half) % 4,
                        single_packet=sp,
                    )
                last_G = G

                if not do_dve:
                    continue
                # one-hot A [128e, col, 64w] via is_equal(tgt_slot, iota)
                ohA = oha_pool.tile([128, NB, W_WIN], BF16, tag="ohA",
                                    name="ohA")
                t3 = tgtA_sb[:, g * NB:(g + 1) * NB].rearrange(
                    "p (b one) -> p b one", one=1)
                i3 = iota_sb[:].rearrange("p (one w) -> p one w", one=1)
                a_bc, b_bc = broadcast_tensor_aps(t3, i3)
                nc.vector.tensor_tensor(out=ohA[:], in0=a_bc, in1=b_bc,
                                        op=ALU.is_equal)
                last_ohA = ohA

                # att_i per bin: onehotB.T @ att_win  (64-contraction)
                S = s_pool.tile([128, NB, HC], BF16, tag="S", name="S")
                NBH = NB // 2
                for hf in range(2 if do_mm else 0):
                    apsg = atti_ps.tile([128, NBH * HC], F32, tag="atti",
                                        name="atti")
                    for c2 in range(NBH):
                        col = hf * NBH + c2
                        w, _j = col_to_wj(col)
                        lwin = g * GW + w
                        nc.tensor.matmul(
                            apsg[:, c2 * HC:(c2 + 1) * HC],
                            ohB[:, col * 128:(col + 1) * 128],
                            attiwin[:, lwin, :],
                            start=True, stop=True, skip_group_check=True,
                        )
                    # s = att_j + att_i
                    nc.vector.tensor_tensor(
                        out=S[:, hf * NBH:(hf + 1) * NBH, :],
                        in0=G[:, hf * NBH:(hf + 1) * NBH, HC:2 * HC],
                        in1=apsg[:].rearrange("p (b c) -> p b c", c=HC),
                        op=ALU.add)
                if not do_mm:
                    nc.vector.tensor_copy(S[:], G[:, :, HC:2 * HC])
                # l = max(0.2*s, s)  (leaky relu)
                nc.vector.scalar_tensor_tensor(
                    out=S[:], in0=S[:], scalar=NEG_SLOPE, in1=S[:],
                    op0=ALU.mult, op1=ALU.max)
                # x = exp(l) -> overwrite att_j half of G
                nc.scalar.activation(G[:, :, HC:2 * HC], S[:], ACT.Exp)
                # y = h * x -> overwrite h half of G
                nc.vector.tensor_tensor(
                    out=G[:, :, 0:HC], in0=G[:, :, 0:HC],
                    in1=G[:, :, HC:2 * HC], op=ALU.mult)

                pair_tiles = {}
                for col in range(NB if do_mm else 0):
                    w, jj = col_to_wj(col)
                    lwin = g * GW + w
                    pr, par = lwin // 2, lwin % 2
                    if jj == 0 and par == 0:
                        pair_tiles[pr] = mm_ps.tile(
                            [128, 2 * HC], F32, tag="pp", name=f"pp{pr % 4}")
                    ps_t = pair_tiles[pr]
                    nc.tensor.matmul(
                        ps_t[HC * par:HC * par + HC, :],
                        ohA[:, col, :],
                        G[:, col, :],
                        start=(jj == 0), stop=(jj == K - 1),
                        tile_position=(0, HC * par),
                        skip_group_check=True,
                    )
                    if jj == K - 1 and par == 1:
                        nc.vector.tensor_copy(
                            stage_n[:, HC * pr:HC * (pr + 1)], ps_t[:, 0:HC])
                        nc.vector.tensor_copy(
                            stage_d[:, HC * pr:HC * (pr + 1)],
                            ps_t[:, HC:2 * HC])
                        del pair_tiles[pr]

            # ---- Phase C: out = numer / (denom + eps) + bias
            if not do_mm:
                nc.vector.memset(stage_n[:], 0.0)
                nc.vector.memset(stage_d[:], 1.0)
            nc.vector.tensor_scalar_add(stage_d[:], stage_d[:], 1e-16)
            lnd = fl_pool.tile([128, NPAIR * HC], F32, tag="lnd", name="lnd")
            nc.scalar.activation(lnd[:], stage_d[:], ACT.Ln)
            nc.scalar.activation(lnd[:], lnd[:], ACT.Exp, scale=-1.0)
            nc.vector.tensor_tensor(out=stage_n[:], in0=stage_n[:], in1=lnd[:],
                                    op=ALU.mult)
            nc.vector.tensor_tensor(out=stage_n[:], in0=stage_n[:],
                                    in1=bias_sb[:], op=ALU.add)

            out_view = out_d[:].rearrange("(pr p) c -> p pr c", p=128)
            st_view = stage_n[:].rearrange("p (pr c) -> p pr c", c=HC)
            nc.sync.dma_start(out_view, st_view)
            return last_G, last_ohA, stage_d

        for rep in range(repeat):
            last_G, last_ohA, stage_d = emit_once(rep)
            if repeat > 1:
                tc.strict_bb_all_engine_barrier()

        if debug_dump:
            dump_hj = nc.dram_tensor("dump_hj", [N_PAD, 2 * HC], BF16,
                                     kind="ExternalOutput")
            dump_aw = nc.dram_tensor("dump_aw", [W_WIN, NPC * HC], BF16,
                                     kind="ExternalOutput")
            dump_sd = nc.dram_tensor("dump_sd", [128, NPAIR * HC], F32,
                                     kind="ExternalOutput")
            dump_g = nc.dram_tensor("dump_g", [128, NB * 2 * HC], BF16,
                                    kind="ExternalOutput")
            dump_oha = nc.dram_tensor("dump_oha", [128, NB * W_WIN], BF16,
                                      kind="ExternalOutput")
            tc.strict_bb_all_engine_barrier()
            nc.sync.dma_start(dump_hj[:], hj_table[:])
            nc.sync.dma_start(dump_aw[:],
                              attiwin[:].rearrange("p a b -> p (a b)"))
            nc.sync.dma_start(dump_sd[:], stage_d[:])
            nc.sync.dma_start(dump_g[:], last_G[:].rearrange("p a b -> p (a b)"))
            nc.sync.dma_start(dump_oha[:],
                              last_ohA[:].rearrange("p a b -> p (a b)"))

    nc.compile()
    return nc


def kernel(**inputs):
    adj = np.asarray(inputs["adj_list"])
    cfg = _choose_cfg(adj[0].astype(np.int64))
    in_maps = _prep(inputs, cfg)
    nc = _build_program(cfg)

    from concourse import bass_utils
    res = bass_utils.run_bass_kernel_spmd(
        nc, in_maps, core_ids=list(range(cfg["NC"])),
        trace=bool(int(os.environ.get("GAT_TRACE", "0"))),
    )
    kernel.last_result = res  # stash for test harness (exec_time_ns etc.)
    kernel.last_ctx = (nc, in_maps, cfg)

    NSHARD = cfg["NSHARD"]
    out_slots = np.zeros((cfg["N_PAD"], HC), dtype=np.float32)
    for c in range(cfg["NC"]):
        out_slots[c * NSHARD:(c + 1) * NSHARD] = res.results[c]["out"]
    return out_slots[cfg["perm"]]


# revision 5
# speedup vs baseline: 99.2723x; 1.0830x over previous
"""Bass/Trainium2 kernel for nn_BitGatConv (GAT-style message passing), v2.

Self-contained: takes full inputs, shards edges by destination window across
8 NeuronCores (SPMD, one program), returns the full [N, HC] output.

v2 changes vs the original baseline:
  - Balanced window packing: nodes are permuted so every 64-node destination
    window has <= K*128 in-edges with K=8 uniform; bin padding drops from
    ~25% to ~2.5%.
  - Overlapped A/B gather views: the int16 index-reach split is handled by
    two OVERLAPPING row views of one hj table ([0, 32768) and
    [N_PAD-32768, N_PAD)); edges with sources in the overlap are assigned to
    whichever half has room, so each window uses exactly 4 A-bins + 4 B-bins.
  - The att_i edge gather (previously 32MB/core of 256B-descriptor DMA) is
    eliminated: per-bin one-hot matrices are generated on device (iota
    is_equal) for the scatter, and att_i[tgt] is computed per bin as a
    64-contraction matmul onehotB.T @ att_win on the PE.

Algorithm (per core, rotated node ids so all cores run the same program):
  Phase A (build): hj = nodes_ft @ [W | W@A2] -> bf16 table [N_PAD, 128] in
    DRAM; att_i windows = nodes_ft @ (W@A1) for the core's own shard kept
    resident in SBUF as [64, NPC, 64] bf16.
  Phase B (edges): per group of 4 windows (32 bins of 128 edge slots):
    gather hj rows by src (two dma_gather calls, A/B views); generate
    onehotA [128e, 64w] via is_equal(tgt_slot, iota); att_i per bin via
    matmul(onehotB [64w,128e], att_win); s = att_i + att_j;
    l = leaky_relu(s); x = exp(l); payload = [x*h | x]; scatter-accumulate
    via matmul(onehotA, payload) into per-window-pair PSUM.
  Phase C (flush): out = numer / (denom + 1e-16) + bias.
  No segment-max subtraction: logits are bounded (~|s|<12) so exp is safe,
  and softmax is shift-free identical.
"""

import math
import os
import sys
from contextlib import ExitStack

import numpy as np

for _p in ("/opt/trn_rl_repo",):
    if _p not in sys.path:
        sys.path.insert(0, _p)

import ml_dtypes  # noqa: E402

BF16_NP = ml_dtypes.bfloat16

# ---------------------------------------------------------------------------
# Problem constants (hardcoded per contest rules)
N_NODES = 50000
N_EDGES = 800000
IN_CH = 128
HC = 64
NEG_SLOPE = 0.2
N_CORES = 8
W_WIN = 64          # nodes per destination window
K_BINS = 8          # bins (of 128 edge slots) per window
JA = 4              # bins gathered from view A (rest from view B)
VIEW = 32768        # rows per gather view (int16 index reach)


def _pack_windows(tgt_global, n_nodes, nw):
    """Balanced-assignment of nodes to nw windows of <=64 nodes, minimizing
    the max window in-degree sum. Zig-zag LPT: process nodes in descending-
    degree chunks of nw, pairing heaviest nodes with lightest windows.
    Returns perm: node -> global slot id (window*64 + position)."""
    deg = np.bincount(tgt_global, minlength=n_nodes).astype(np.int64)
    order = np.argsort(-deg, kind="stable")
    weights = np.zeros(nw, dtype=np.int64)
    counts = np.zeros(nw, dtype=np.int64)
    win_of_node = np.empty(n_nodes, dtype=np.int64)
    pos = 0
    while pos < n_nodes:
        chunk = order[pos:pos + nw]
        worder = np.argsort(weights, kind="stable")  # lightest first
        wsel = worder[: len(chunk)]
        win_of_node[chunk] = wsel
        np.add.at(weights, wsel, deg[chunk])
        np.add.at(counts, wsel, 1)
        pos += nw
    assert counts.max() <= W_WIN
    # slot within window by arrival order
    slot_in_win = np.zeros(n_nodes, dtype=np.int64)
    occupied = np.zeros(nw, dtype=np.int64)
    for nd in order:  # deterministic fill
        w = win_of_node[nd]
        slot_in_win[nd] = occupied[w]
        occupied[w] += 1
    perm = win_of_node * W_WIN + slot_in_win
    return perm, int(weights.max())


def _cfg(npc):
    nw = N_CORES * npc
    n_pad = nw * W_WIN
    nshard = npc * W_WIN
    gw = 4 if npc % 4 == 0 else 3
    assert npc % gw == 0 and npc % 2 == 0
    return dict(
        N=N_NODES, E=N_EDGES, NC=N_CORES, W=W_WIN, NPC=npc, NW=nw,
        N_PAD=n_pad, NSHARD=nshard, GROUP_NW=gw, K=K_BINS, JA=JA,
        B=npc * K_BINS, NB=gw * K_BINS, NGROUPS=npc // gw,
        NPAIR=npc // 2, BOFF=n_pad - VIEW,
        T_TILES=n_pad // 128, SHARD_TILES=nshard // 128,
    )


def _choose_cfg(tgt_global):
    for npc in (100, 104, 108):
        cfg = _cfg(npc)
        perm, maxw = _pack_windows(tgt_global, N_NODES, cfg["NW"])
        if maxw <= K_BINS * 128:
            cfg["perm"] = perm
            return cfg
    raise AssertionError(f"window packing failed (maxw={maxw})")


def _wrap16(stream2d):
    # [NC, L] -> [NC, 128, L//16] in dma_gather wrapped layout
    # (idx i -> [i%16, i//16], replicated 8x across partition groups)
    ncc, L = stream2d.shape
    w = stream2d.reshape(ncc, L // 16, 16).transpose(0, 2, 1)
    return np.ascontiguousarray(np.tile(w, (1, 8, 1)))


def _prep(inputs, cfg):
    """Host-side preprocessing: pack + shard + index/stream building."""
    NC, W, NPC = cfg["NC"], cfg["W"], cfg["NPC"]
    N_PAD, NSHARD, BOFF = cfg["N_PAD"], cfg["NSHARD"], cfg["BOFF"]
    K, GW, B, NB = cfg["K"], cfg["GROUP_NW"], cfg["B"], cfg["NB"]
    NG = cfg["NGROUPS"]

    nodes_ft = np.asarray(inputs["nodes_ft"], dtype=np.float32)
    adj = np.asarray(inputs["adj_list"])
    weight = np.asarray(inputs["weight"], dtype=np.float32)
    a1 = np.asarray(inputs["att_layer_1"], dtype=np.float32)
    a2 = np.asarray(inputs["att_layer_2"], dtype=np.float32)
    bias = np.asarray(inputs["bias"], dtype=np.float32)

    tgt = adj[0].astype(np.int64)
    src = adj[1].astype(np.int64)
    perm = cfg["perm"]

    tslot = perm[tgt]                      # global dst slot
    sslot = perm[src]                      # global src slot
    core = tslot // NSHARD
    lw = (tslot % NSHARD) // W             # local window 0..NPC-1
    wslot = tslot % W                      # slot within window 0..63
    srot = (sslot - core * NSHARD) % N_PAD

    # class: 0 = A-forced (srot < BOFF), 1 = free, 2 = B-forced (>= VIEW)
    cls = np.where(srot < BOFF, 0, np.where(srot < VIEW, 1, 2))

    wkey = core * NPC + lw                 # global window id (core-major)
    NWIN = NC * NPC
    order1 = np.argsort(wkey * 4 + cls, kind="stable")
    eo_w = wkey[order1]
    cnt = np.bincount(wkey, minlength=NWIN)
    cntA = np.bincount(wkey[cls == 0], minlength=NWIN)
    starts = np.zeros(NWIN + 1, dtype=np.int64)
    starts[1:] = np.cumsum(cnt)
    CAP = JA * 128
    assert cnt.max() <= K * 128, cnt.max()
    assert cntA.max() <= CAP, cntA.max()
    assert np.bincount(wkey[cls == 2], minlength=NWIN).max() <= CAP
    nA = cntA + np.maximum(0, cnt - CAP - cntA)   # A-set size per window
    assert (nA <= CAP).all() and (cnt - nA <= CAP).all()

    r = np.arange(cfg["E"], dtype=np.int64) - starts[eo_w]
    inA1 = r < nA[eo_w]
    halfid = np.empty(cfg["E"], dtype=np.int64)   # original-edge space
    halfid[order1] = (~inA1).astype(np.int64)     # 0 = A view, 1 = B view

    # final slot order: within (window, half) sort by source row so gather
    # descriptors walk HBM mostly monotonically (row locality)
    order = np.lexsort((srot, halfid, wkey))
    grp2 = wkey[order] * 2 + halfid[order]
    cnt2 = np.bincount(grp2, minlength=NWIN * 2)
    starts2 = np.zeros(NWIN * 2 + 1, dtype=np.int64)
    starts2[1:] = np.cumsum(cnt2)
    rnk = np.arange(cfg["E"], dtype=np.int64) - starts2[grp2]
    inA = halfid[order] == 0
    j = rnk // 128                          # bin index within half (0..3)
    p = rnk % 128
    assert j.max() < JA

    c_e = core[order]
    lw_e = lw[order]
    g_e = lw_e // GW
    wl_e = lw_e % GW
    # bin column within group (G-tile order): A bins 0..GW*JA-1, then B bins
    colg = np.where(inA, wl_e * JA + j, GW * JA + wl_e * JA + j)
    b_core = g_e * NB + colg                # bin id within core 0..B-1

    # gather index streams (per view), flat per core
    LH = NG * (GW * JA) * 128               # idx per core per view
    hjA = np.zeros((NC, LH), dtype=np.int16)
    hjB = np.zeros((NC, LH), dtype=np.int16)
    iposA = (g_e * (GW * JA) + wl_e * JA + j) * 128 + p
    mA = inA
    hjA[c_e[mA], iposA[mA]] = srot[order][mA].astype(np.int16)
    mB = ~inA
    hjB[c_e[mB], iposA[mB]] = (srot[order][mB] - BOFF).astype(np.int16)

    # tgt slot stream [NC, 128, B] bf16 (64.0 = pad -> zero one-hot column)
    tgtA = np.full((NC, 128, B), np.float32(W), dtype=np.float32)
    tgtA[c_e, p, b_core] = wslot[order].astype(np.float32)

    # one-hot B stream [NC, 64, B*128] bf16
    ohB = np.zeros((NC, W, B * 128), dtype=np.float32)
    ohB[c_e, wslot[order], b_core * 128 + p] = 1.0

    # constants
    whwj = np.concatenate([weight, weight @ a2], axis=1).astype(BF16_NP)
    wi = (weight @ a1).astype(BF16_NP)
    iota64 = np.tile(np.arange(W, dtype=np.float32), (128, 1)).astype(BF16_NP)
    NPAIR = cfg["NPAIR"]
    bias_full = np.tile(bias[None, :], (128, NPAIR)).astype(np.float32)

    # permuted, transposed, padded node features (bf16)
    base = np.zeros((IN_CH, N_PAD), dtype=np.float32)
    base[:, perm] = nodes_ft.T

    in_maps = []
    for c in range(NC):
        nftT = np.ascontiguousarray(np.roll(base, -c * NSHARD, axis=1))
        in_maps.append({
            "nodes_ftT": nftT.astype(BF16_NP),
            "whwj": whwj, "wi": wi,
            "iota64": iota64,
            "hjA_idx": _wrap16(hjA[c:c + 1])[0],
            "hjB_idx": _wrap16(hjB[c:c + 1])[0],
            "tgtA": tgtA[c].astype(BF16_NP),
            "onehotB": ohB[c].astype(BF16_NP),
            "bias_bc": bias_full,
        })
    return in_maps


def _build_program(cfg, debug_dump=False, phase_limit="full", repeat=1,
                   single_packet=None):
    import concourse.bacc as bacc
    import concourse.bass as bass
    import concourse.mybir as mybir
    import concourse.tile as tile
    from concourse.bass import broadcast_tensor_aps

    BF16 = mybir.dt.bfloat16
    F32 = mybir.dt.float32
    I16 = mybir.dt.int16
    ALU = mybir.AluOpType
    ACT = mybir.ActivationFunctionType

    NPC, N_PAD, NSHARD = cfg["NPC"], cfg["N_PAD"], cfg["NSHARD"]
    T_TILES, SHARD_TILES = cfg["T_TILES"], cfg["SHARD_TILES"]
    GW, K, B, NB, NG = (cfg["GROUP_NW"], cfg["K"], cfg["B"], cfg["NB"],
                        cfg["NGROUPS"])
    NPAIR, BOFF = cfg["NPAIR"], cfg["BOFF"]
    NBA = GW * JA                      # A bins per group
    LH = NG * NBA * 128                # gather idx per core per view

    nc = bacc.Bacc("TRN2", target_bir_lowering=False, debug=False,
                   num_swdge_queues=4)

    nodes_ftT = nc.dram_tensor("nodes_ftT", [IN_CH, N_PAD], BF16, kind="ExternalInput")
    whwj_d = nc.dram_tensor("whwj", [IN_CH, 2 * HC], BF16, kind="ExternalInput")
    wi_d = nc.dram_tensor("wi", [IN_CH, HC], BF16, kind="ExternalInput")
    iota_d = nc.dram_tensor("iota64", [128, W_WIN], BF16, kind="ExternalInput")
    hjA_d = nc.dram_tensor("hjA_idx", [128, LH // 16], I16, kind="ExternalInput")
    hjB_d = nc.dram_tensor("hjB_idx", [128, LH // 16], I16, kind="ExternalInput")
    tgtA_d = nc.dram_tensor("tgtA", [128, B], BF16, kind="ExternalInput")
    ohB_d = nc.dram_tensor("onehotB", [W_WIN, B * 128], BF16, kind="ExternalInput")
    bias_d = nc.dram_tensor("bias_bc", [128, NPAIR * HC], F32, kind="ExternalInput")
    out_d = nc.dram_tensor("out", [NSHARD, HC], F32, kind="ExternalOutput")

    hj_table = nc.dram_tensor("hj_table", [N_PAD, 2 * HC], BF16, kind="Internal")

    do_build = phase_limit != "noop"
    do_gather = phase_limit in ("gather", "nomm", "full")
    do_dve = phase_limit in ("nomm", "full")
    do_mm = phase_limit == "full"

    with tile.TileContext(nc) as tc, ExitStack() as ctx:
        const_pool = ctx.enter_context(tc.tile_pool(name="const", bufs=1))
        b_in = ctx.enter_context(tc.tile_pool(name="b_in", bufs=4))
        b_ps = ctx.enter_context(tc.tile_pool(name="b_ps", bufs=2, space="PSUM"))
        aw_ps = ctx.enter_context(tc.tile_pool(name="aw_ps", bufs=1, space="PSUM"))
        b_st = ctx.enter_context(tc.tile_pool(name="b_st", bufs=4))
        idx_pool = ctx.enter_context(tc.tile_pool(name="idx", bufs=6))
        g_pool = ctx.enter_context(tc.tile_pool(name="gp", bufs=3))
        ohb_pool = ctx.enter_context(tc.tile_pool(name="ohb", bufs=3))
        oha_pool = ctx.enter_context(tc.tile_pool(name="oha", bufs=2))
        s_pool = ctx.enter_context(tc.tile_pool(name="sp", bufs=2))
        atti_ps = ctx.enter_context(tc.tile_pool(name="attips", bufs=1, space="PSUM"))
        mm_ps = ctx.enter_context(tc.tile_pool(name="mmps", bufs=2, space="PSUM"))
        fl_pool = ctx.enter_context(tc.tile_pool(name="fl", bufs=1))

        whwj_sb = const_pool.tile([IN_CH, 2 * HC], BF16)
        nc.sync.dma_start(whwj_sb[:], whwj_d[:])
        wi_sb = const_pool.tile([IN_CH, HC], BF16)
        nc.sync.dma_start(wi_sb[:], wi_d[:])
        iota_sb = const_pool.tile([128, W_WIN], BF16)
        nc.sync.dma_start(iota_sb[:], iota_d[:])
        tgtA_sb = const_pool.tile([128, B], BF16)
        nc.sync.dma_start(tgtA_sb[:], tgtA_d[:])
        bias_sb = const_pool.tile([128, NPAIR * HC], F32)
        nc.sync.dma_start(bias_sb[:], bias_d[:])
        attiwin = const_pool.tile([W_WIN, NPC, HC], BF16)

        def emit_once(rep):
            # ---- Phase A: build hj table (DRAM) + att_i windows (SBUF)
            # Each iteration handles 512 nodes (fewer dma_start issues: the
            # ~0.5-1us HWDGE issue cost per DMA dominates Phase A otherwise).
            # The four matmuls split nodes mod 4 so store-partition p holds
            # nodes {4p..4p+3}: the table write is 1KB-contiguous per
            # partition (full DMA rate; 256B-row writes would eat the
            # small-descriptor RMW penalty).
            for q in range(T_TILES // 4 if do_build else 0):
                dmae = nc.sync if q % 2 == 0 else nc.scalar
                nf = b_in.tile([128, 512], BF16, name="nf")
                dmae.dma_start(nf[:], nodes_ftT[:, 512 * q:512 * (q + 1)])
                nfv = nf[:].rearrange("p (n four) -> p four n", four=4)
                ps = b_ps.tile([128, 4, 2 * HC], F32, name="bps")
                for u in range(4):
                    nc.tensor.matmul(ps[:, u, :], nfv[:, u, :], whwj_sb[:],
                                     start=True, stop=True)
                st = b_st.tile([128, 4, 2 * HC], BF16, name="bst")
                if q % 2 == 0:
                    nc.vector.tensor_copy(st[:], ps[:])
                else:
                    nc.scalar.copy(st[:], ps[:])
                dmae.dma_start(
                    hj_table[512 * q:512 * (q + 1), :].rearrange(
                        "(p four) b -> p four b", p=128),
                    st[:])
                for wv in range(8):
                    # att_i for the core's own shard -> resident SBUF windows
                    w = 8 * q + wv
                    if w >= NPC:
                        break
                    aps = aw_ps.tile([W_WIN, HC], F32, tag=f"aw{wv % 2}",
                                     name="aw")
                    nc.tensor.matmul(
                        aps[:], nf[:, 64 * wv:64 * (wv + 1)],
                        wi_sb[:], start=True, stop=True)
                    if wv % 2 == 0:
                        nc.vector.tensor_copy(attiwin[:, w, :], aps[:])
                    else:
                        nc.scalar.copy(attiwin[:, w, :], aps[:])

            # ---- Phase B: edge processing
            stage_n = fl_pool.tile([128, NPAIR * HC], F32, tag="sn", name="sn")
            stage_d = fl_pool.tile([128, NPAIR * HC], F32, tag="sd", name="sd")

            def col_to_wj(col):
                if col < NBA:
                    return col // JA, col % JA
                c2 = col - NBA
                return c2 // JA, JA + c2 % JA

            last_G = last_ohA = None
            for g in range(NG if do_gather else 0):
                sl = idx_pool.tile([128, NBA * 8], I16, tag="sl", name="sl")
                nc.sync.dma_start(sl[:], hjA_d[:, g * NBA * 8:(g + 1) * NBA * 8])
                sh = idx_pool.tile([128, NBA * 8], I16, tag="sh", name="sh")
                nc.sync.dma_start(sh[:], hjB_d[:, g * NBA * 8:(g + 1) * NBA * 8])
                ohB = ohb_pool.tile([W_WIN, NB * 128], BF16, tag="ohB",
                                    name="ohB")
                # all Phase B DMA issue stays on the sync queue: the scalar
                # (ACT) queue runs exp, and a dma_start queued behind exp
                # would stall the next group's gather pipeline.
                nc.sync.dma_start(ohB[:], ohB_d[:, g * NB * 128:(g + 1) * NB * 128])

                G = g_pool.tile([128, NB, 2 * HC], BF16, tag="G", name="G")
                sp = (bool(int(os.environ.get("GAT_SINGLE_PACKET", "0")))
                      if single_packet is None else single_packet)
                for half, idxs, lo in ((0, sl, 0), (1, sh, BOFF)):
                    nc.gpsimd.dma_gather(
                        out_ap=G[:, half * NBA:(half + 1) * NBA, :],
                        in_ap=hj_table[lo:lo + VIEW, :],
                        idxs_ap=idxs[:],
                        num_idxs=NBA * 128,
                        num_idxs_reg=NBA * 128,
                        elem_size=2 * HC, queue_num=(2 * g + half) % 4,
                        single_packet=sp,
                    )
                last_G = G

                if not do_dve:
                    continue
                # one-hot A [128e, col, 64w] via is_equal(tgt_slot, iota)
                ohA = oha_pool.tile([128, NB, W_WIN], BF16, tag="ohA",
                                    name="ohA")
                t3 = tgtA_sb[:, g * NB:(g + 1) * NB].rearrange(
                    "p (b one) -> p b one", one=1)
                i3 = iota_sb[:].rearrange("p (one w) -> p one w", one=1)
                a_bc, b_bc = broadcast_tensor_aps(t3, i3)
                nc.vector.tensor_tensor(out=ohA[:], in0=a_bc, in1=b_bc,
                                        op=ALU.is_equal)
                last_ohA = ohA

                # att_i per bin: onehotB.T @ att_win  (64-contraction)
                S = s_pool.tile([128, NB, HC], BF16, tag="S", name="S")
                NBH = NB // 2
                for hf in range(2 if do_mm else 0):
                    apsg = atti_ps.tile([128, NBH * HC], F32, tag="atti",
                                        name="atti")
                    for c2 in range(NBH):
                        col = hf * NBH + c2
                        w, _j = col_to_wj(col)
                        lwin = g * GW + w
                        nc.tensor.matmul(
                            apsg[:, c2 * HC:(c2 + 1) * HC],
                            ohB[:, col * 128:(col + 1) * 128],
                            attiwin[:, lwin, :],
                            start=True, stop=True, skip_group_check=True,
                        )
                    # s = att_j + att_i
                    nc.vector.tensor_tensor(
                        out=S[:, hf * NBH:(hf + 1) * NBH, :],
                        in0=G[:, hf * NBH:(hf + 1) * NBH, HC:2 * HC],
                        in1=apsg[:].rearrange("p (b c) -> p b c", c=HC),
                        op=ALU.add)
                if not do_mm:
                    nc.vector.tensor_copy(S[:], G[:, :, HC:2 * HC])
                # l = max(0.2*s, s)  (leaky relu)
                nc.vector.scalar_tensor_tensor(
                    out=S[:], in0=S[:], scalar=NEG_SLOPE, in1=S[:],
                    op0=ALU.mult, op1=ALU.max)
                # x = exp(l) -> overwrite att_j half of G
                nc.scalar.activation(G[:, :, HC:2 * HC], S[:], ACT.Exp)
                # y = h * x -> overwrite h half of G
                nc.vector.tensor_tensor(
                    out=G[:, :, 0:HC], in0=G[:, :, 0:HC],
                    in1=G[:, :, HC:2 * HC], op=ALU.mult)

                pair_tiles = {}
                for col in range(NB if do_mm else 0):
                    w, jj = col_to_wj(col)
                    lwin = g * GW + w
                    pr, par = lwin // 2, lwin % 2
                    if jj == 0 and par == 0:
                        pair_tiles[pr] = mm_ps.tile(
                            [128, 2 * HC], F32, tag="pp", name=f"pp{pr % 4}")
                    ps_t = pair_tiles[pr]
                    nc.tensor.matmul(
                        ps_t[HC * par:HC * par + HC, :],
                        ohA[:, col, :],
                        G[:, col, :],
                        start=(jj == 0), stop=(jj == K - 1),
                        tile_position=(0, HC * par),
                        skip_group_check=True,
                    )
                    if jj == K - 1 and par == 1:
                        nc.vector.tensor_copy(
                            stage_n[:, HC * pr:HC * (pr + 1)], ps_t[:, 0:HC])
                        nc.vector.tensor_copy(
                            stage_d[:, HC * pr:HC * (pr + 1)],
                            ps_t[:, HC:2 * HC])
                        del pair_tiles[pr]

            # ---- Phase C: out = numer / (denom + eps) + bias
            if not do_mm:
                nc.vector.memset(stage_n[:], 0.0)
                nc.vector.memset(stage_d[:], 1.0)
            nc.vector.tensor_scalar_add(stage_d[:], stage_d[:], 1e-16)
            lnd = fl_pool.tile([128, NPAIR * HC], F32, tag="lnd", name="lnd")
            nc.scalar.activation(lnd[:], stage_d[:], ACT.Ln)
            nc.scalar.activation(lnd[:], lnd[:], ACT.Exp, scale=-1.0)
            nc.vector.tensor_tensor(out=stage_n[:], in0=stage_n[:], in1=lnd[:],
                                    op=ALU.mult)
            nc.vector.tensor_tensor(out=stage_n[:], in0=stage_n[:],
                                    in1=bias_sb[:], op=ALU.add)

            out_view = out_d[:].rearrange("(pr p) c -> p pr c", p=128)
            st_view = stage_n[:].rearrange("p (pr c) -> p pr c", c=HC)
            nc.sync.dma_start(out_view, st_view)
            return last_G, last_ohA, stage_d

        for rep in range(repeat):
            last_G, last_ohA, stage_d = emit_once(rep)
            if repeat > 1:
                tc.strict_bb_all_engine_barrier()

        if debug_dump:
            dump_hj = nc.dram_tensor("dump_hj", [N_PAD, 2 * HC], BF16,
                                     kind="ExternalOutput")
            dump_aw = nc.dram_tensor("dump_aw", [W_WIN, NPC * HC], BF16,
                                     kind="ExternalOutput")
            dump_sd = nc.dram_tensor("dump_sd", [128, NPAIR * HC], F32,
                                     kind="ExternalOutput")
            dump_g = nc.dram_tensor("dump_g", [128, NB * 2 * HC], BF16,
                                    kind="ExternalOutput")
            dump_oha = nc.dram_tensor("dump_oha", [128, NB * W_WIN], BF16,
                                      kind="ExternalOutput")
            tc.strict_bb_all_engine_barrier()
            nc.sync.dma_start(dump_hj[:], hj_table[:])
            nc.sync.dma_start(dump_aw[:],
                              attiwin[:].rearrange("p a b -> p (a b)"))
            nc.sync.dma_start(dump_sd[:], stage_d[:])
            nc.sync.dma_start(dump_g[:], last_G[:].rearrange("p a b -> p (a b)"))
            nc.sync.dma_start(dump_oha[:],
                              last_ohA[:].rearrange("p a b -> p (a b)"))

    nc.compile()
    return nc


def kernel(**inputs):
    adj = np.asarray(inputs["adj_list"])
    cfg = _choose_cfg(adj[0].astype(np.int64))
    in_maps = _prep(inputs, cfg)
    nc = _build_program(cfg)

    from concourse import bass_utils
    res = bass_utils.run_bass_kernel_spmd(
        nc, in_maps, core_ids=list(range(cfg["NC"])),
        trace=bool(int(os.environ.get("GAT_TRACE", "0"))),
    )
    kernel.last_result = res  # stash for test harness (exec_time_ns etc.)
    kernel.last_ctx = (nc, in_maps, cfg)

    NSHARD = cfg["NSHARD"]
    out_slots = np.zeros((cfg["N_PAD"], HC), dtype=np.float32)
    for c in range(cfg["NC"]):
        out_slots[c * NSHARD:(c + 1) * NSHARD] = res.results[c]["out"]
    return out_slots[cfg["perm"]]


# revision 6
# speedup vs baseline: 121.3473x; 1.2224x over previous
"""Bass/Trainium2 kernel for nn_BitGatConv (GAT-style message passing), v2.

Self-contained: takes full inputs, shards edges by destination window across
8 NeuronCores (SPMD, one program), returns the full [N, HC] output.

v2 changes vs the original baseline:
  - Balanced window packing: nodes are permuted so every 64-node destination
    window has <= K*128 in-edges with K=8 uniform; bin padding drops from
    ~25% to ~2.5%.
  - Overlapped A/B gather views: the int16 index-reach split is handled by
    two OVERLAPPING row views of one hj table ([0, 32768) and
    [N_PAD-32768, N_PAD)); edges with sources in the overlap are assigned to
    whichever half has room, so each window uses exactly 4 A-bins + 4 B-bins.
  - The att_i edge gather (previously 32MB/core of 256B-descriptor DMA) is
    eliminated: per-bin one-hot matrices are generated on device (iota
    is_equal) for the scatter, and att_i[tgt] is computed per bin as a
    64-contraction matmul onehotB.T @ att_win on the PE.

Algorithm (per core, rotated node ids so all cores run the same program):
  Phase A (build): hj = nodes_ft @ [W | W@A2] -> bf16 table [N_PAD, 128] in
    DRAM; att_i windows = nodes_ft @ (W@A1) for the core's own shard kept
    resident in SBUF as [64, NPC, 64] bf16.
  Phase B (edges): per group of 4 windows (32 bins of 128 edge slots):
    gather hj rows by src (two dma_gather calls, A/B views); generate
    onehotA [128e, 64w] via is_equal(tgt_slot, iota); att_i per bin via
    matmul(onehotB [64w,128e], att_win); s = att_i + att_j;
    l = leaky_relu(s); x = exp(l); payload = [x*h | x]; scatter-accumulate
    via matmul(onehotA, payload) into per-window-pair PSUM.
  Phase C (flush): out = numer / (denom + 1e-16) + bias.
  No segment-max subtraction: logits are bounded (~|s|<12) so exp is safe,
  and softmax is shift-free identical.
"""

import math
import os
import sys
from contextlib import ExitStack

import numpy as np

for _p in ("/opt/trn_rl_repo",):
    if _p not in sys.path:
        sys.path.insert(0, _p)

import ml_dtypes  # noqa: E402

BF16_NP = ml_dtypes.bfloat16

# ---------------------------------------------------------------------------
# Problem constants (hardcoded per contest rules)
N_NODES = 50000
N_EDGES = 800000
IN_CH = 128
HC = 64
NEG_SLOPE = 0.2
N_CORES = 8
W_WIN = 64          # nodes per destination window
K_BINS = 8          # bins (of 128 edge slots) per window
JA = 4              # bins gathered from view A (rest from view B)
VIEW = 32768        # rows per gather view (int16 index reach)


def _pack_windows(tgt_global, n_nodes, nw):
    """Balanced-assignment of nodes to nw windows of <=64 nodes, minimizing
    the max window in-degree sum. Zig-zag LPT: process nodes in descending-
    degree chunks of nw, pairing heaviest nodes with lightest windows.
    Returns perm: node -> global slot id (window*64 + position)."""
    deg = np.bincount(tgt_global, minlength=n_nodes).astype(np.int64)
    order = np.argsort(-deg, kind="stable")
    weights = np.zeros(nw, dtype=np.int64)
    counts = np.zeros(nw, dtype=np.int64)
    win_of_node = np.empty(n_nodes, dtype=np.int64)
    pos = 0
    while pos < n_nodes:
        chunk = order[pos:pos + nw]
        worder = np.argsort(weights, kind="stable")  # lightest first
        wsel = worder[: len(chunk)]
        win_of_node[chunk] = wsel
        np.add.at(weights, wsel, deg[chunk])
        np.add.at(counts, wsel, 1)
        pos += nw
    assert counts.max() <= W_WIN
    # slot within window by arrival order
    slot_in_win = np.zeros(n_nodes, dtype=np.int64)
    occupied = np.zeros(nw, dtype=np.int64)
    for nd in order:  # deterministic fill
        w = win_of_node[nd]
        slot_in_win[nd] = occupied[w]
        occupied[w] += 1
    perm = win_of_node * W_WIN + slot_in_win
    return perm, int(weights.max())


def _cfg(npc):
    nw = N_CORES * npc
    n_pad = nw * W_WIN
    nshard = npc * W_WIN
    gw = 4 if npc % 4 == 0 else 3
    assert npc % gw == 0 and npc % 2 == 0
    return dict(
        N=N_NODES, E=N_EDGES, NC=N_CORES, W=W_WIN, NPC=npc, NW=nw,
        N_PAD=n_pad, NSHARD=nshard, GROUP_NW=gw, K=K_BINS, JA=JA,
        B=npc * K_BINS, NB=gw * K_BINS, NGROUPS=npc // gw,
        NPAIR=npc // 2, BOFF=n_pad - VIEW,
        T_TILES=n_pad // 128, SHARD_TILES=nshard // 128,
    )


def _choose_cfg(tgt_global):
    for npc in (100, 104, 108):
        cfg = _cfg(npc)
        perm, maxw = _pack_windows(tgt_global, N_NODES, cfg["NW"])
        if maxw <= K_BINS * 128:
            cfg["perm"] = perm
            return cfg
    raise AssertionError(f"window packing failed (maxw={maxw})")


def _wrap16(stream2d):
    # [NC, L] -> [NC, 128, L//16] in dma_gather wrapped layout
    # (idx i -> [i%16, i//16], replicated 8x across partition groups)
    ncc, L = stream2d.shape
    w = stream2d.reshape(ncc, L // 16, 16).transpose(0, 2, 1)
    return np.ascontiguousarray(np.tile(w, (1, 8, 1)))


def _prep(inputs, cfg):
    """Host-side preprocessing: pack + shard + index/stream building."""
    NC, W, NPC = cfg["NC"], cfg["W"], cfg["NPC"]
    N_PAD, NSHARD, BOFF = cfg["N_PAD"], cfg["NSHARD"], cfg["BOFF"]
    K, GW, B, NB = cfg["K"], cfg["GROUP_NW"], cfg["B"], cfg["NB"]
    NG = cfg["NGROUPS"]

    nodes_ft = np.asarray(inputs["nodes_ft"], dtype=np.float32)
    adj = np.asarray(inputs["adj_list"])
    weight = np.asarray(inputs["weight"], dtype=np.float32)
    a1 = np.asarray(inputs["att_layer_1"], dtype=np.float32)
    a2 = np.asarray(inputs["att_layer_2"], dtype=np.float32)
    bias = np.asarray(inputs["bias"], dtype=np.float32)

    tgt = adj[0].astype(np.int64)
    src = adj[1].astype(np.int64)
    perm = cfg["perm"]

    tslot = perm[tgt]                      # global dst slot
    sslot = perm[src]                      # global src slot
    core = tslot // NSHARD
    lw = (tslot % NSHARD) // W             # local window 0..NPC-1
    wslot = tslot % W                      # slot within window 0..63
    srot = (sslot - core * NSHARD) % N_PAD

    # class: 0 = A-forced (srot < BOFF), 1 = free, 2 = B-forced (>= VIEW)
    cls = np.where(srot < BOFF, 0, np.where(srot < VIEW, 1, 2))

    wkey = core * NPC + lw                 # global window id (core-major)
    NWIN = NC * NPC
    order1 = np.argsort(wkey * 4 + cls, kind="stable")
    eo_w = wkey[order1]
    cnt = np.bincount(wkey, minlength=NWIN)
    cntA = np.bincount(wkey[cls == 0], minlength=NWIN)
    starts = np.zeros(NWIN + 1, dtype=np.int64)
    starts[1:] = np.cumsum(cnt)
    CAP = JA * 128
    assert cnt.max() <= K * 128, cnt.max()
    assert cntA.max() <= CAP, cntA.max()
    assert np.bincount(wkey[cls == 2], minlength=NWIN).max() <= CAP
    nA = cntA + np.maximum(0, cnt - CAP - cntA)   # A-set size per window
    assert (nA <= CAP).all() and (cnt - nA <= CAP).all()

    r = np.arange(cfg["E"], dtype=np.int64) - starts[eo_w]
    inA1 = r < nA[eo_w]
    halfid = np.empty(cfg["E"], dtype=np.int64)   # original-edge space
    halfid[order1] = (~inA1).astype(np.int64)     # 0 = A view, 1 = B view

    # final slot order: within (window, half) sort by source row so gather
    # descriptors walk HBM mostly monotonically (row locality)
    order = np.lexsort((srot, halfid, wkey))
    grp2 = wkey[order] * 2 + halfid[order]
    cnt2 = np.bincount(grp2, minlength=NWIN * 2)
    starts2 = np.zeros(NWIN * 2 + 1, dtype=np.int64)
    starts2[1:] = np.cumsum(cnt2)
    rnk = np.arange(cfg["E"], dtype=np.int64) - starts2[grp2]
    inA = halfid[order] == 0
    j = rnk // 128                          # bin index within half (0..3)
    p = rnk % 128
    assert j.max() < JA

    c_e = core[order]
    lw_e = lw[order]
    g_e = lw_e // GW
    wl_e = lw_e % GW
    # bin column within group (G-tile order): A bins 0..GW*JA-1, then B bins
    colg = np.where(inA, wl_e * JA + j, GW * JA + wl_e * JA + j)
    b_core = g_e * NB + colg                # bin id within core 0..B-1

    # gather index streams (per view), flat per core
    LH = NG * (GW * JA) * 128               # idx per core per view
    hjA = np.zeros((NC, LH), dtype=np.int16)
    hjB = np.zeros((NC, LH), dtype=np.int16)
    iposA = (g_e * (GW * JA) + wl_e * JA + j) * 128 + p
    mA = inA
    hjA[c_e[mA], iposA[mA]] = srot[order][mA].astype(np.int16)
    mB = ~inA
    hjB[c_e[mB], iposA[mB]] = (srot[order][mB] - BOFF).astype(np.int16)

    # tgt slot stream [NC, 128, B] bf16 (64.0 = pad -> zero one-hot column)
    tgtA = np.full((NC, 128, B), np.float32(W), dtype=np.float32)
    tgtA[c_e, p, b_core] = wslot[order].astype(np.float32)

    # one-hot B stream [NC, 64, B*128] bf16
    ohB = np.zeros((NC, W, B * 128), dtype=np.float32)
    ohB[c_e, wslot[order], b_core * 128 + p] = 1.0

    # constants
    whwj = np.concatenate([weight, weight @ a2], axis=1).astype(BF16_NP)
    wi = (weight @ a1).astype(BF16_NP)
    iota64 = np.tile(np.arange(W, dtype=np.float32), (128, 1)).astype(BF16_NP)
    NPAIR = cfg["NPAIR"]
    bias_full = np.tile(bias[None, :], (128, NPAIR)).astype(np.float32)

    # permuted, transposed, padded node features (bf16)
    base = np.zeros((IN_CH, N_PAD), dtype=np.float32)
    base[:, perm] = nodes_ft.T

    in_maps = []
    for c in range(NC):
        nftT = np.ascontiguousarray(np.roll(base, -c * NSHARD, axis=1))
        in_maps.append({
            "nodes_ftT": nftT.astype(BF16_NP),
            "whwj": whwj, "wi": wi,
            "iota64": iota64,
            "hjA_idx": _wrap16(hjA[c:c + 1])[0],
            "hjB_idx": _wrap16(hjB[c:c + 1])[0],
            "tgtA": tgtA[c].astype(BF16_NP),
            "onehotB": ohB[c].astype(BF16_NP),
            "bias_bc": bias_full,
        })
    return in_maps


def _build_program(cfg, debug_dump=False, phase_limit="full", repeat=1,
                   single_packet=None):
    import concourse.bacc as bacc
    import concourse.bass as bass
    import concourse.mybir as mybir
    import concourse.tile as tile
    from concourse.bass import broadcast_tensor_aps

    BF16 = mybir.dt.bfloat16
    F32 = mybir.dt.float32
    I16 = mybir.dt.int16
    ALU = mybir.AluOpType
    ACT = mybir.ActivationFunctionType

    NPC, N_PAD, NSHARD = cfg["NPC"], cfg["N_PAD"], cfg["NSHARD"]
    T_TILES, SHARD_TILES = cfg["T_TILES"], cfg["SHARD_TILES"]
    GW, K, B, NB, NG = (cfg["GROUP_NW"], cfg["K"], cfg["B"], cfg["NB"],
                        cfg["NGROUPS"])
    NPAIR, BOFF = cfg["NPAIR"], cfg["BOFF"]
    NBA = GW * JA                      # A bins per group
    LH = NG * NBA * 128                # gather idx per core per view

    nc = bacc.Bacc("TRN2", target_bir_lowering=False, debug=False,
                   num_swdge_queues=4)

    nodes_ftT = nc.dram_tensor("nodes_ftT", [IN_CH, N_PAD], BF16, kind="ExternalInput")
    whwj_d = nc.dram_tensor("whwj", [IN_CH, 2 * HC], BF16, kind="ExternalInput")
    wi_d = nc.dram_tensor("wi", [IN_CH, HC], BF16, kind="ExternalInput")
    iota_d = nc.dram_tensor("iota64", [128, W_WIN], BF16, kind="ExternalInput")
    hjA_d = nc.dram_tensor("hjA_idx", [128, LH // 16], I16, kind="ExternalInput")
    hjB_d = nc.dram_tensor("hjB_idx", [128, LH // 16], I16, kind="ExternalInput")
    tgtA_d = nc.dram_tensor("tgtA", [128, B], BF16, kind="ExternalInput")
    ohB_d = nc.dram_tensor("onehotB", [W_WIN, B * 128], BF16, kind="ExternalInput")
    bias_d = nc.dram_tensor("bias_bc", [128, NPAIR * HC], F32, kind="ExternalInput")
    out_d = nc.dram_tensor("out", [NSHARD, HC], F32, kind="ExternalOutput")

    hj_table = nc.dram_tensor("hj_table", [N_PAD, 2 * HC], BF16, kind="Internal")

    do_build = phase_limit != "noop"
    do_gather = phase_limit in ("gather", "nomm", "full")
    do_dve = phase_limit in ("nomm", "full")
    do_mm = phase_limit == "full"

    with tile.TileContext(nc) as tc, ExitStack() as ctx:
        const_pool = ctx.enter_context(tc.tile_pool(name="const", bufs=1))
        b_in = ctx.enter_context(tc.tile_pool(name="b_in", bufs=4))
        b_ps = ctx.enter_context(tc.tile_pool(name="b_ps", bufs=2, space="PSUM"))
        aw_ps = ctx.enter_context(tc.tile_pool(name="aw_ps", bufs=1, space="PSUM"))
        b_st = ctx.enter_context(tc.tile_pool(name="b_st", bufs=4))
        idx_pool = ctx.enter_context(tc.tile_pool(name="idx", bufs=8))
        g_pool = ctx.enter_context(tc.tile_pool(name="gp", bufs=4))
        ohb_pool = ctx.enter_context(tc.tile_pool(name="ohb", bufs=4))
        oha_pool = ctx.enter_context(tc.tile_pool(name="oha", bufs=2))
        s_pool = ctx.enter_context(tc.tile_pool(name="sp", bufs=2))
        atti_ps = ctx.enter_context(tc.tile_pool(name="attips", bufs=1, space="PSUM"))
        mm_ps = ctx.enter_context(tc.tile_pool(name="mmps", bufs=2, space="PSUM"))
        fl_pool = ctx.enter_context(tc.tile_pool(name="fl", bufs=1))

        whwj_sb = const_pool.tile([IN_CH, 2 * HC], BF16)
        nc.sync.dma_start(whwj_sb[:], whwj_d[:])
        wi_sb = const_pool.tile([IN_CH, HC], BF16)
        nc.sync.dma_start(wi_sb[:], wi_d[:])
        iota_sb = const_pool.tile([128, W_WIN], BF16)
        nc.sync.dma_start(iota_sb[:], iota_d[:])
        tgtA_sb = const_pool.tile([128, B], BF16)
        nc.sync.dma_start(tgtA_sb[:], tgtA_d[:])
        bias_sb = const_pool.tile([128, NPAIR * HC], F32)
        nc.sync.dma_start(bias_sb[:], bias_d[:])
        attiwin = const_pool.tile([W_WIN, NPC, HC], BF16)

        def emit_once(rep):
            # ---- Phase A: build hj table (DRAM) + att_i windows (SBUF)
            # Each iteration handles 512 nodes (fewer dma_start issues: the
            # ~0.5-1us HWDGE issue cost per DMA dominates Phase A otherwise).
            # The four matmuls split nodes mod 4 so store-partition p holds
            # nodes {4p..4p+3}: the table write is 1KB-contiguous per
            # partition (full DMA rate; 256B-row writes would eat the
            # small-descriptor RMW penalty).
            for q in range(T_TILES // 4 if do_build else 0):
                dmae = nc.sync if q % 2 == 0 else nc.scalar
                nf = b_in.tile([128, 512], BF16, name="nf")
                dmae.dma_start(nf[:], nodes_ftT[:, 512 * q:512 * (q + 1)])
                nfv = nf[:].rearrange("p (n four) -> p four n", four=4)
                ps = b_ps.tile([128, 4, 2 * HC], F32, name="bps")
                for u in range(4):
                    nc.tensor.matmul(ps[:, u, :], nfv[:, u, :], whwj_sb[:],
                                     start=True, stop=True)
                st = b_st.tile([128, 4, 2 * HC], BF16, name="bst")
                if q % 2 == 0:
                    nc.vector.tensor_copy(st[:], ps[:])
                else:
                    nc.scalar.copy(st[:], ps[:])
                dmae.dma_start(
                    hj_table[512 * q:512 * (q + 1), :].rearrange(
                        "(p four) b -> p four b", p=128),
                    st[:])
                for wv in range(8):
                    # att_i for the core's own shard -> resident SBUF windows
                    w = 8 * q + wv
                    if w >= NPC:
                        break
                    aps = aw_ps.tile([W_WIN, HC], F32, tag=f"aw{wv % 2}",
                                     name="aw")
                    nc.tensor.matmul(
                        aps[:], nf[:, 64 * wv:64 * (wv + 1)],
                        wi_sb[:], start=True, stop=True)
                    if wv % 2 == 0:
                        nc.vector.tensor_copy(attiwin[:, w, :], aps[:])
                    else:
                        nc.scalar.copy(attiwin[:, w, :], aps[:])

            # ---- Phase B: edge processing
            stage_n = fl_pool.tile([128, NPAIR * HC], F32, tag="sn", name="sn")
            stage_d = fl_pool.tile([128, NPAIR * HC], F32, tag="sd", name="sd")

            def col_to_wj(col):
                if col < NBA:
                    return col // JA, col % JA
                c2 = col - NBA
                return c2 // JA, JA + c2 % JA

            last_G = last_ohA = None
            for g in range(NG if do_gather else 0):
                sl = idx_pool.tile([128, NBA * 8], I16, tag="sl", name="sl")
                nc.sync.dma_start(sl[:], hjA_d[:, g * NBA * 8:(g + 1) * NBA * 8])
                sh = idx_pool.tile([128, NBA * 8], I16, tag="sh", name="sh")
                nc.sync.dma_start(sh[:], hjB_d[:, g * NBA * 8:(g + 1) * NBA * 8])
                ohB = ohb_pool.tile([W_WIN, NB * 128], BF16, tag="ohB",
                                    name="ohB")
                # all Phase B DMA issue stays on the sync queue: the scalar
                # (ACT) queue runs exp, and a dma_start queued behind exp
                # would stall the next group's gather pipeline.
                nc.sync.dma_start(ohB[:], ohB_d[:, g * NB * 128:(g + 1) * NB * 128])

                G = g_pool.tile([128, NB, 2 * HC], BF16, tag="G", name="G")
                sp = (bool(int(os.environ.get("GAT_SINGLE_PACKET", "0")))
                      if single_packet is None else single_packet)
                for half, idxs, lo in ((0, sl, 0), (1, sh, BOFF)):
                    nc.gpsimd.dma_gather(
                        out_ap=G[:, half * NBA:(half + 1) * NBA, :],
                        in_ap=hj_table[lo:lo + VIEW, :],
                        idxs_ap=idxs[:],
                        num_idxs=NBA * 128,
                        num_idxs_reg=NBA * 128,
                        elem_size=2 * HC, queue_num=(2 * g + half) % 4,
                        single_packet=sp,
                    )
                last_G = G

                if not do_dve:
                    continue
                # one-hot A [128e, col, 64w] via is_equal(tgt_slot, iota)
                ohA = oha_pool.tile([128, NB, W_WIN], BF16, tag="ohA",
                                    name="ohA")
                t3 = tgtA_sb[:, g * NB:(g + 1) * NB].rearrange(
                    "p (b one) -> p b one", one=1)
                i3 = iota_sb[:].rearrange("p (one w) -> p one w", one=1)
                a_bc, b_bc = broadcast_tensor_aps(t3, i3)
                nc.vector.tensor_tensor(out=ohA[:], in0=a_bc, in1=b_bc,
                                        op=ALU.is_equal)
                last_ohA = ohA

                # att_i per bin: onehotB.T @ att_win  (64-contraction)
                S = s_pool.tile([128, NB, HC], BF16, tag="S", name="S")
                NBH = NB // 2
                for hf in range(2 if do_mm else 0):
                    apsg = atti_ps.tile([128, NBH * HC], F32, tag="atti",
                                        name="atti")
                    for c2 in range(NBH):
                        col = hf * NBH + c2
                        w, _j = col_to_wj(col)
                        lwin = g * GW + w
                        nc.tensor.matmul(
                            apsg[:, c2 * HC:(c2 + 1) * HC],
                            ohB[:, col * 128:(col + 1) * 128],
                            attiwin[:, lwin, :],
                            start=True, stop=True, skip_group_check=True,
                        )
                    # s = att_j + att_i
                    nc.vector.tensor_tensor(
                        out=S[:, hf * NBH:(hf + 1) * NBH, :],
                        in0=G[:, hf * NBH:(hf + 1) * NBH, HC:2 * HC],
                        in1=apsg[:].rearrange("p (b c) -> p b c", c=HC),
                        op=ALU.add)
                if not do_mm:
                    nc.vector.tensor_copy(S[:], G[:, :, HC:2 * HC])
                # l = max(0.2*s, s)  (leaky relu)
                nc.vector.scalar_tensor_tensor(
                    out=S[:], in0=S[:], scalar=NEG_SLOPE, in1=S[:],
                    op0=ALU.mult, op1=ALU.max)
                # x = exp(l) -> overwrite att_j half of G
                nc.scalar.activation(G[:, :, HC:2 * HC], S[:], ACT.Exp)
                # y = h * x -> overwrite h half of G
                nc.vector.tensor_tensor(
                    out=G[:, :, 0:HC], in0=G[:, :, 0:HC],
                    in1=G[:, :, HC:2 * HC], op=ALU.mult)

                pair_tiles = {}
                for col in range(NB if do_mm else 0):
                    w, jj = col_to_wj(col)
                    lwin = g * GW + w
                    pr, par = lwin // 2, lwin % 2
                    if jj == 0 and par == 0:
                        pair_tiles[pr] = mm_ps.tile(
                            [128, 2 * HC], F32, tag="pp", name=f"pp{pr % 4}")
                    ps_t = pair_tiles[pr]
                    nc.tensor.matmul(
                        ps_t[HC * par:HC * par + HC, :],
                        ohA[:, col, :],
                        G[:, col, :],
                        start=(jj == 0), stop=(jj == K - 1),
                        tile_position=(0, HC * par),
                        skip_group_check=True,
                    )
                    if jj == K - 1 and par == 1:
                        nc.vector.tensor_copy(
                            stage_n[:, HC * pr:HC * (pr + 1)], ps_t[:, 0:HC])
                        nc.vector.tensor_copy(
                            stage_d[:, HC * pr:HC * (pr + 1)],
                            ps_t[:, HC:2 * HC])
                        del pair_tiles[pr]

            # ---- Phase C: out = numer / (denom + eps) + bias
            if not do_mm:
                nc.vector.memset(stage_n[:], 0.0)
                nc.vector.memset(stage_d[:], 1.0)
            nc.vector.tensor_scalar_add(stage_d[:], stage_d[:], 1e-16)
            lnd = fl_pool.tile([128, NPAIR * HC], F32, tag="lnd", name="lnd")
            nc.scalar.activation(lnd[:], stage_d[:], ACT.Ln)
            nc.scalar.activation(lnd[:], lnd[:], ACT.Exp, scale=-1.0)
            nc.vector.tensor_tensor(out=stage_n[:], in0=stage_n[:], in1=lnd[:],
                                    op=ALU.mult)
            nc.vector.tensor_tensor(out=stage_n[:], in0=stage_n[:],
                                    in1=bias_sb[:], op=ALU.add)

            out_view = out_d[:].rearrange("(pr p) c -> p pr c", p=128)
            st_view = stage_n[:].rearrange("p (pr c) -> p pr c", c=HC)
            nc.sync.dma_start(out_view, st_view)
            return last_G, last_ohA, stage_d

        for rep in range(repeat):
            last_G, last_ohA, stage_d = emit_once(rep)
            if repeat > 1:
                tc.strict_bb_all_engine_barrier()

        if debug_dump:
            dump_hj = nc.dram_tensor("dump_hj", [N_PAD, 2 * HC], BF16,
                                     kind="ExternalOutput")
            dump_aw = nc.dram_tensor("dump_aw", [W_WIN, NPC * HC], BF16,
                                     kind="ExternalOutput")
            dump_sd = nc.dram_tensor("dump_sd", [128, NPAIR * HC], F32,
                                     kind="ExternalOutput")
            dump_g = nc.dram_tensor("dump_g", [128, NB * 2 * HC], BF16,
                                    kind="ExternalOutput")
            dump_oha = nc.dram_tensor("dump_oha", [128, NB * W_WIN], BF16,
                                      kind="ExternalOutput")
            tc.strict_bb_all_engine_barrier()
            nc.sync.dma_start(dump_hj[:], hj_table[:])
            nc.sync.dma_start(dump_aw[:],
                              attiwin[:].rearrange("p a b -> p (a b)"))
            nc.sync.dma_start(dump_sd[:], stage_d[:])
            nc.sync.dma_start(dump_g[:], last_G[:].rearrange("p a b -> p (a b)"))
            nc.sync.dma_start(dump_oha[:],
                              last_ohA[:].rearrange("p a b -> p (a b)"))

    nc.compile()
    return nc


def kernel(**inputs):
    adj = np.asarray(inputs["adj_list"])
    cfg = _choose_cfg(adj[0].astype(np.int64))
    in_maps = _prep(inputs, cfg)
    nc = _build_program(cfg)

    from concourse import bass_utils
    res = bass_utils.run_bass_kernel_spmd(
        nc, in_maps, core_ids=list(range(cfg["NC"])),
        trace=bool(int(os.environ.get("GAT_TRACE", "0"))),
    )
    kernel.last_result = res  # stash for test harness (exec_time_ns etc.)
    kernel.last_ctx = (nc, in_maps, cfg)

    NSHARD = cfg["NSHARD"]
    out_slots = np.zeros((cfg["N_PAD"], HC), dtype=np.float32)
    for c in range(cfg["NC"]):
        out_slots[c * NSHARD:(c + 1) * NSHARD] = res.results[c]["out"]
    return out_slots[cfg["perm"]]
